# revision 33
# baseline (speedup 1.0000x reference)
"""Trainium2 Bass kernel for nn_Block_27848567948000 (dense transformer block).

Sharding (8 NeuronCores): 4 data-parallel groups over batch (B=4), 2-way
tensor-parallel within each pair: attention sharded over heads (5 each).
out_proj computed as per-head partial sums over ALL T, summed + token-scattered
via pairwise ReduceScatters; MLP over the core's T/2 tokens.

Token ownership (per pair): rank0 owns tiles {0-3, 8-11}, rank1 owns
{4-7, 12-15}, so ReduceScatter #1 (rows 0:1024 = tiles 0-7) can fire right
after query-block qb=1 and RS#2 after qb=3; fc1 on the first half overlaps
RS#2.

Quantization: the reference's mxfp8 QDQ equals a plain e4m3 cast under a
global power-of-2 scale for all values in fp8-normal range (validated
numerically: rel err ~2e-3 incl. fp8 softmax probs).  q/k are quantized with
scale 32*rstd (rms fold), v with 32; w_out pre-scaled by 1/32 on the host.
NOTE dt.float8e4 is IEEE e4m3: max finite 240, inf above -- scale 32 keeps
|q*32|<=170, |v*32|<=196, and exp bias -1 keeps p<=80.  Softmax probs are fp8,
enabling DoubleRow (2x fp8) matmuls for P@V + denominator over kt-tile pairs.

kernel(**inputs) takes FULL inputs and returns the FULL (4, 2048, 1280) output.
"""
import sys

sys.path.insert(0, '/opt/trn_rl_repo')

import numpy as np
import ml_dtypes

import concourse.bass as bass
import concourse.tile as tile
from concourse import mybir, bacc
from concourse import bass_utils
from concourse.masks import make_identity

B, T, C, H, D, F = 4, 2048, 1280, 10, 128, 5120
EPS = 1e-5
N_CORES = 8
HPC = H // 2            # heads per core (5)
CPC = HPC * D           # channels per core (640)
f32 = mybir.dt.float32
bf16 = mybir.dt.bfloat16
fp8 = mybir.dt.float8e4
i32 = mybir.dt.int32
AF = mybir.ActivationFunctionType
OP = mybir.AluOpType
AX = mybir.AxisListType
DR = mybir.MatmulPerfMode.DoubleRow

NT = T // 128            # 16 token tiles
NH = T // 2 // 128       # 8 token tiles in my half
QB = T // 512            # 4 query blocks
INV_SQRT_D = float(1.0 / np.sqrt(D))
EXP_BIAS = -1.0
QSCALE = 32.0   # global fp8 scale for q/k/v (e4m3 max finite is 240!)


def _rsqrt_vec(nc, pool, out_ap, in_ap, scale, eps, tag, eng=None):
    """out = 1/sqrt(in*scale + eps) on a DVE-like engine (no act tables).
    Bit-trick seed + 2 Newton iterations (~1e-6 rel err). Shapes (128, n)."""
    if eng is None:
        eng = nc.vector
    i32_ = mybir.dt.int32
    shp = [128, in_ap.free_size()]
    m = pool.tile(shp, f32, tag=tag + 'm', name='rs_m')
    eng.tensor_scalar(out=m[:], in0=in_ap, scalar1=scale, scalar2=eps,
                      op0=OP.mult, op1=OP.add)
    y = pool.tile(shp, f32, tag=tag + 'y', name='rs_y')
    eng.tensor_single_scalar(out=y[:].bitcast(i32_),
                             in_=m[:].bitcast(i32_), scalar=1,
                             op=OP.logical_shift_right)
    eng.tensor_scalar(out=y[:].bitcast(i32_), in0=y[:].bitcast(i32_),
                      scalar1=-1, scalar2=0x5f3759df,
                      op0=OP.mult, op1=OP.add)
    t = pool.tile(shp, f32, tag=tag + 't', name='rs_t')
    for it in range(2):
        eng.tensor_tensor(out=t[:], in0=y[:], in1=y[:], op=OP.mult)
        eng.tensor_tensor(out=t[:], in0=t[:], in1=m[:], op=OP.mult)
        eng.tensor_scalar(out=t[:], in0=t[:], scalar1=-0.5,
                          scalar2=1.5, op0=OP.mult, op1=OP.add)
        eng.tensor_tensor(out=y[:] if it == 0 else out_ap, in0=y[:],
                          in1=t[:], op=OP.mult)


def _ap(t_ap, offset_delta, pattern):
    return bass.AP(tensor=t_ap.tensor, offset=t_ap.offset + offset_delta,
                   ap=pattern)


def build_nc(t_len=T, n_cores=N_CORES):
    import contextlib
    nc = bacc.Bacc('TRN2', target_bir_lowering=False, debug=False,
                   num_devices=n_cores)

    # ---- DRAM I/O ----
    # xsq: row-major bf16 x (for rmsnorm sum-of-squares)
    # xT:  transposed bf16 x [C, T] (QKV lhsT; no PE transposes needed)
    xsq_d = nc.dram_tensor('xsq', [T, C], bf16, kind='ExternalInput')
    xT_d = nc.dram_tensor('xT', [C, T], fp8, kind='ExternalInput')
    xh_d = nc.dram_tensor('xh', [T // 2, C], f32, kind='ExternalInput')
    wqkv_d = nc.dram_tensor('w_qkv', [C, 3 * CPC], fp8, kind='ExternalInput')
    cossin_d = nc.dram_tensor('cossin', [T, 4 * D], bf16,
                              kind='ExternalInput')
    wout_d = nc.dram_tensor('w_out', [CPC, C], fp8, kind='ExternalInput')
    wfc1_d = nc.dram_tensor('w_fc1', [C, F], bf16, kind='ExternalInput')
    wfc2_d = nc.dram_tensor('w_fc2', [F, C], bf16, kind='ExternalInput')
    y_d = nc.dram_tensor('y', [T // 2, C], f32, kind='ExternalOutput')

    with tile.TileContext(nc) as tc:
        with contextlib.ExitStack() as ctx:
            persist = ctx.enter_context(tc.tile_pool(name='persist', bufs=1))
            dram = ctx.enter_context(tc.tile_pool(name='dram', bufs=1,
                                                  space='DRAM'))

            # ---- constants ----
            identb = persist.tile([128, 128], bf16)
            make_identity(nc, identb)
            ones256 = persist.tile([128, 256], fp8)
            nc.vector.memset(ones256[:], 1.0)
            zero_sb = persist.tile([128, 1], f32)
            nc.vector.memset(zero_sb[:], 0.0)
            ebias_sb = persist.tile([128, 1], f32)
            nc.vector.memset(ebias_sb[:], EXP_BIAS)
            scr_sq = persist.tile([128, C], bf16)   # Square-output scratch
            # residual x2 for my first 4 tiles, staged during qb3 so phase D
            # can start fc1 immediately (overlapping RS#2)
            x2a = persist.tile([128, 4, C], f32)
            x2b4 = persist.tile([128, 4, C], bf16)

            # DRAM scratch for the collective
            rs_in = dram.tile([T, C], bf16)
            rs_out = dram.tile([T // 2, C], bf16)

            with contextlib.ExitStack() as pab:
                ab = pab.enter_context(tc.tile_pool(name='ab', bufs=1))
                qT = ab.tile([128, HPC, T], bf16)
                kT = ab.tile([128, HPC, T], bf16)
                vd_sb = ab.tile([128, NT, HPC, D], fp8)
                at_p = pab.enter_context(tc.tile_pool(name='at_p', bufs=2))

                # ====== phases A+B ======
                with contextlib.ExitStack() as pin:
                    a_w = pin.enter_context(tc.tile_pool(name='a_w', bufs=1))
                    a_x = pin.enter_context(tc.tile_pool(name='a_x', bufs=3))
                    wq_sb = a_w.tile([128, 10, 3 * CPC], fp8)
                    cs_sb = a_w.tile([128, NT, 4, D], bf16)

                    def load_aw():
                        nc.sync.dma_start(
                            out=wq_sb[:],
                            in_=wqkv_d.ap().rearrange('(j p) c -> p j c',
                                                      p=128))
                        nc.sync.dma_start(
                            out=cs_sb[:],
                            in_=cossin_d.ap().rearrange('(t p) x -> p t x',
                                                        p=128))

                    a_t = pin.enter_context(tc.tile_pool(name='a_t', bufs=2))
                    a_s = pin.enter_context(tc.tile_pool(name='a_s', bufs=2))
                    a_q = pin.enter_context(tc.tile_pool(name='a_q', bufs=4))
                    pT_pool = pin.enter_context(
                        tc.tile_pool(name='pT', bufs=4))
                    b_t = pin.enter_context(tc.tile_pool(name='b_t', bufs=2))
                    ps512 = pin.enter_context(
                        tc.tile_pool(name='ps512', bufs=3, space='PSUM'))
                    ops_ps = pin.enter_context(
                        tc.tile_pool(name='ops_ps', bufs=2, space='PSUM'))
                    psT = pin.enter_context(
                        tc.tile_pool(name='psT', bufs=2, space='PSUM'))
                    psD = pin.enter_context(
                        tc.tile_pool(name='psD', bufs=1, space='PSUM'))

                    stash = {}

                    def emit_head(t, first=False):
                        # rstd of x for this token tile (from bf16 x rows)
                        xt = a_s.tile([128, C], bf16, tag='xt')
                        nc.sync.dma_start(
                            out=xt[:], in_=xsq_d[t * 128:(t + 1) * 128, :])
                        xTt = a_x.tile([128, 10, 128], fp8, tag='xTt')
                        nc.sync.dma_start(
                            out=xTt[:],
                            in_=xT_d[:, t * 128:(t + 1) * 128]
                            .rearrange('(j p) t -> p j t', p=128))
                        if first:
                            load_aw()
                        ssq = a_s.tile([128, 1], f32, tag='ssq')
                        nc.scalar.activation(out=scr_sq[:], in_=xt[:],
                                             func=AF.Square, bias=zero_sb[:],
                                             accum_out=ssq[:])
                        # rstd scaled by 2^-15 to undo x*32 and w*1024 fp8
                        # scaling: rsqrt((ssq/C + EPS) * 2^30)
                        rstd = a_s.tile([128, 1], f32, tag='rstd')
                        _rsqrt_vec(nc, a_s, rstd[:], ssq[:],
                                   float((2.0 ** 30) / C),
                                   float(EPS * (2.0 ** 30)), 'rx')
                        # QKV (chunk-outer, j-mid, g-inner: LDW amortized)
                        qf = a_q.tile([128, CPC], bf16, tag='qf')
                        kf = a_q.tile([128, CPC], bf16, tag='kf')
                        vf = a_q.tile([128, CPC], bf16, tag='vf')
                        dsts = (qf, kf, vf)
                        for lo, hi in ((0, 512), (512, 640)):
                            pss = [ps512.tile([128, 512], f32, tag='mm',
                                              name='qkvps')
                                   for _ in range(3)]
                            for jp in range(5):
                                for g in range(3):
                                    nc.tensor.matmul(
                                        pss[g][:, 0:hi - lo],
                                        xTt[:, 2 * jp:2 * jp + 2, :],
                                        wq_sb[:, 2 * jp:2 * jp + 2,
                                              g * CPC + lo:g * CPC + hi],
                                        start=(jp == 0), stop=(jp == 4),
                                        perf_mode=DR)
                            for g in range(3):
                                if g == 0:
                                    nc.vector.tensor_scalar_mul(
                                        out=dsts[g][:, lo:hi],
                                        in0=pss[g][:, 0:hi - lo],
                                        scalar1=rstd[:])
                                else:
                                    nc.scalar.activation(
                                        out=dsts[g][:, lo:hi],
                                        in_=pss[g][:, 0:hi - lo],
                                        func=AF.Copy, scale=rstd[:])
                        stash[t] = (qf, kf, vf)

                    def rope(eng, src, cos_t, sin_t, out):
                        # out[p,h,d] = src*cos + swap(src)*sinneg   (bf16)
                        src3 = src[:].rearrange('p (h d) -> p h d', h=HPC)
                        pa = list(src3.ap)
                        swap = _ap(src3, 64, pa[:2] + [[-64, 2], [1, 64]])
                        ca = list(cos_t.ap)
                        cos4 = _ap(cos_t, 0, [ca[0], [0, HPC], [1, 128]])
                        sin4 = _ap(sin_t, 0,
                                   [ca[0], [0, HPC], [64, 2], [1, 64]])
                        tmp = a_t.tile([128, HPC, D], bf16, tag='rtmp')
                        eng.tensor_tensor(
                            out=tmp[:].rearrange('p h (u d) -> p h u d', u=2),
                            in0=swap, in1=sin4, op=OP.mult)
                        eng.tensor_tensor(out=out[:], in0=src3, in1=cos4,
                                          op=OP.mult)
                        eng.tensor_add(out=out[:], in0=out[:], in1=tmp[:])

                    def hb(ap5, reps):
                        # (128,5) -> (128,5,reps) broadcast
                        a = list(ap5.ap)
                        return bass.AP(tensor=ap5.tensor, offset=ap5.offset,
                                       ap=[a[0], [a[-1][0], HPC], [0, reps]])

                    def emit_tail(t):
                        qf, kf, vf = stash.pop(t)
                        # rms of pre-rope q/k (rope is norm-preserving);
                        # msq64 = 64 * rstd (the fp8 global scale folded in)
                        msq = a_t.tile([128, 2, HPC], f32, tag='msq')
                        qsq = a_t.tile([128, HPC, D], bf16, tag='qsq')
                        qf3 = qf[:].rearrange('p (h d) -> p h d', h=HPC)
                        nc.vector.tensor_tensor(out=qsq[:], in0=qf3,
                                                in1=qf3, op=OP.mult)
                        nc.vector.tensor_reduce(out=msq[:, 0, :],
                                                in_=qsq[:], axis=AX.X,
                                                op=OP.add)
                        ksq = a_t.tile([128, HPC, D], bf16, tag='ksq')
                        kf3 = kf[:].rearrange('p (h d) -> p h d', h=HPC)
                        nc.vector.tensor_tensor(out=ksq[:], in0=kf3,
                                                in1=kf3, op=OP.mult)
                        nc.vector.tensor_reduce(out=msq[:, 1, :],
                                                in_=ksq[:], axis=AX.X,
                                                op=OP.add)
                        _rsqrt_vec(nc, a_t, msq[:], msq[:],
                                   float(1.0 / (D * QSCALE * QSCALE)),
                                   float(EPS / (QSCALE * QSCALE)), 'rqk')
                        # rope (q on vector, k on gpsimd)
                        zq = a_t.tile([128, HPC, D], bf16, tag='zq')
                        rope(nc.vector, qf, cs_sb[:, t, 0, :],
                             cs_sb[:, t, 1, :], zq)
                        zk = a_t.tile([128, HPC, D], bf16, tag='zk')
                        rope(nc.vector, kf, cs_sb[:, t, 2, :],
                             cs_sb[:, t, 3, :], zk)
                        # quantize: one op per tensor (global-scale e4m3)
                        q8v = a_t.tile([128, HPC, D], fp8, tag='q8')
                        nc.vector.tensor_tensor(out=q8v[:], in0=zq[:],
                                                in1=hb(msq[:, 0, :], D),
                                                op=OP.mult)
                        k8v = a_t.tile([128, HPC, D], fp8, tag='k8')
                        nc.vector.tensor_tensor(out=k8v[:], in0=zk[:],
                                                in1=hb(msq[:, 1, :], D),
                                                op=OP.mult)
                        nc.vector.tensor_scalar_mul(
                            out=vd_sb[:, t, :, :],
                            in0=vf[:].rearrange('p (h d) -> p h d', h=HPC),
                            scalar1=QSCALE)
                        # cast quantized q/k back to bf16 (exact) for the
                        # PE transposes (walrus rejects fp8 transposes)
                        qdb = a_t.tile([128, HPC, D], bf16, tag='qdb')
                        nc.scalar.copy(out=qdb[:], in_=q8v[:])
                        kdb = a_t.tile([128, HPC, D], bf16, tag='kdb')
                        nc.scalar.copy(out=kdb[:], in_=k8v[:])
                        for src, dstT in ((qdb, qT), (kdb, kT)):
                            tp = psT.tile([128, 640], bf16, tag='tp')
                            for h in range(HPC):
                                nc.tensor.transpose(
                                    tp[:, h * 128:(h + 1) * 128],
                                    src[:, h, :], identb[:])
                            nc.vector.tensor_copy(
                                out=dstT[:, :, t * 128:(t + 1) * 128],
                                in_=tp[:].rearrange('p (h d) -> p h d',
                                                    h=HPC))

                    def emit_attn_h(qb, h, at):
                        nkt = 4 * qb + 4
                        dps = psD.tile([128, 512], f32, tag='dps')
                        ops = ops_ps.tile([128, 512], f32, tag='ops')
                        for kp in range(nkt // 2):
                            pT2 = pT_pool.tile([128, 2, 512], fp8, tag='pT')
                            for u in range(2):
                                kt = 2 * kp + u
                                sp = ps512.tile([128, 512], f32, tag='mm')
                                nc.tensor.matmul(
                                    sp[:],
                                    kT[:, h, kt * 128:(kt + 1) * 128],
                                    qT[:, h, qb * 512:(qb + 1) * 512],
                                    start=True, stop=True)
                                nc.scalar.activation(
                                    out=pT2[:, u, :], in_=sp[:],
                                    func=AF.Exp, bias=ebias_sb[:],
                                    scale=float(INV_SQRT_D /
                                                (QSCALE * QSCALE)))
                                o = kt - 4 * qb
                                if o >= 0:
                                    nc.gpsimd.affine_select(
                                        out=pT2[:, u, :], in_=pT2[:, u, :],
                                        compare_op=OP.is_ge, fill=0.0,
                                        base=-128 * o, pattern=[[1, 512]],
                                        channel_multiplier=-1)
                            st = (kp == 0)
                            sp_ = (kp == nkt // 2 - 1)
                            nc.tensor.matmul(
                                dps[:],
                                ones256[:].rearrange('p (u m) -> p u m', u=2),
                                pT2[:], start=st, stop=sp_, perf_mode=DR)
                            nc.tensor.matmul(
                                ops[:], vd_sb[:, 2 * kp:2 * kp + 2, h, :],
                                pT2[:], start=st, stop=sp_, perf_mode=DR)
                        rd = b_t.tile([128, 512], f32, tag='rd')
                        nc.vector.reciprocal_approx_fast(out=rd[:],
                                                         in_=dps[:])
                        nc.vector.tensor_tensor(
                            out=at[:, h, :],
                            in0=ops[:], in1=rd[:], op=OP.mult)

                    wo_sb = a_w.tile([128, HPC, C], fp8)
                    nc.sync.dma_start(
                        out=wo_sb[:],
                        in_=wout_d.ap().rearrange('(h p) c -> p h c', p=128))
                    grp = [[2 * i, 2 * i + 1] for i in range(n_cores // 2)]

                    OSC = float(2.0 ** -15)   # undo attn*32 and wo*1024

                    def oproj(tt, at):
                        tl = (tt % 4) * 128
                        ob = b_t.tile([128, C], bf16, tag='ob', name='ob')
                        for ci, (lo, hi) in enumerate(((0, 512),
                                                       (512, 1024),
                                                       (1024, C))):
                            ps = ps512.tile([128, 512], f32, tag='mm',
                                            name='oprojps')
                            for hp2 in range(2):
                                nc.tensor.matmul(
                                    ps[:, 0:hi - lo],
                                    at[:, 2 * hp2:2 * hp2 + 2, tl:tl + 128],
                                    wo_sb[:, 2 * hp2:2 * hp2 + 2, lo:hi],
                                    start=(hp2 == 0), stop=False,
                                    perf_mode=DR)
                            nc.tensor.matmul(
                                ps[:, 0:hi - lo], at[:, 4, tl:tl + 128],
                                wo_sb[:, 4, lo:hi],
                                start=False, stop=True)
                            if ci == 2:
                                nc.scalar.activation(
                                    out=ob[:, lo:hi],
                                    in_=ps[:, 0:hi - lo],
                                    func=AF.Copy, scale=OSC)
                            else:
                                nc.vector.tensor_scalar_mul(
                                    out=ob[:, lo:hi],
                                    in0=ps[:, 0:hi - lo], scalar1=OSC)
                        nc.sync.dma_start(
                            out=rs_in[tt * 128:(tt + 1) * 128, :], in_=ob[:])

                    # ---- interleaved A+B+C emission ----
                    # attn/oproj units of query-block qb are dripped across
                    # the next 4 head/tail slots so the PE FIFO always has
                    # QKV work queued behind exp-gated score tiles.
                    # RS#1 (tiles 0-7) fires after qb=1's oproj; RS#2 at end.
                    pending = []

                    def drip(n):
                        for _ in range(n):
                            if pending:
                                pending.pop(0)()

                    def mk_attn(qb, h, at):
                        return lambda: emit_attn_h(qb, h, at)

                    def mk_oproj(tt, at):
                        return lambda: oproj(tt, at)

                    def mk_rs1():
                        def f():
                            nc.gpsimd.collective_compute(
                                'ReduceScatter', OP.add,
                                ins=[rs_in[0:1024, :].opt()],
                                outs=[rs_out[0:512, :].opt()],
                                replica_groups=grp)
                        return f

                    for t in range(NT):
                        emit_head(t, first=(t == 0))
                        if t >= 1:
                            emit_tail(t - 1)
                        drip(3)
                        if t >= 4 and t % 4 == 0:
                            qb = t // 4 - 1
                            at = at_p.tile([128, HPC, 512], fp8, tag='at')
                            for h in range(HPC):
                                pending.append(mk_attn(qb, h, at))
                            if qb == 2:
                                # RS#1 trigger goes on the gpsimd queue; it
                                # must come after qb2's affine_selects there
                                # or they stall behind the CC completion.
                                pending.append(mk_rs1())
                            for tt in range(4 * qb, 4 * qb + 4):
                                pending.append(mk_oproj(tt, at))
                    def mlp_pre(tt):
                        # x2 = rs_out + xh for my tiles 0-3 (RS#1 data)
                        rsx = b_t.tile([128, C], bf16, tag='rsx')
                        nc.sync.dma_start(
                            out=rsx[:],
                            in_=rs_out[tt * 128:(tt + 1) * 128, :])
                        xht = b_t.tile([128, C], f32, tag='xht')
                        nc.sync.dma_start(
                            out=xht[:],
                            in_=xh_d[tt * 128:(tt + 1) * 128, :])
                        nc.vector.tensor_add(out=x2a[:, tt, :],
                                             in0=rsx[:], in1=xht[:])
                        nc.vector.tensor_copy(out=x2b4[:, tt, :],
                                              in_=x2a[:, tt, :])

                    emit_tail(NT - 1)
                    drip(len(pending))
                    at = at_p.tile([128, HPC, 512], fp8, tag='at')
                    for h in range(HPC):
                        emit_attn_h(QB - 1, h, at)
                        if h < 4:
                            mlp_pre(h)
                    for tt in range(12, 16):
                        oproj(tt, at)
                    nc.gpsimd.collective_compute(
                        'ReduceScatter', OP.add,
                        ins=[rs_in[1024:2048, :].opt()],
                        outs=[rs_out[512:1024, :].opt()],
                        replica_groups=grp)

            # ====== phase D: residual + MLP over my T/2 tokens ======
            # half 0 (tiles 0-3) depends only on RS#1 -> overlaps RS#2.
            with contextlib.ExitStack() as pd:
                d_t = pd.enter_context(tc.tile_pool(name='d_t', bufs=2))
                d_big = pd.enter_context(tc.tile_pool(name='d_big', bufs=1))
                x2_sb = d_big.tile([128, 4, C], f32)
                xn2T = d_big.tile([128, 10, T // 2], bf16)
                h2T = d_big.tile([128, F // 128, T // 2], bf16)
                rinv_sb = d_big.tile([128, NH], f32)

                with tc.tile_pool(name='d_ps', bufs=4, space='PSUM') as d_ps, \
                     tc.tile_pool(name='dt_ps', bufs=2,
                                  space='PSUM') as dt_ps, \
                     tc.tile_pool(name='wf1', bufs=5) as wf1_pool:

                    def x2ap(tt):
                        return x2a[:, tt, :] if tt < 4 \
                            else x2_sb[:, tt - 4, :]

                    def mlp_head(tt):
                        # x2 = rs_out + xh; rstd; transposes into xn2T
                        if tt < 4:
                            x2b = x2b4[:, tt, :]
                        else:
                            rsx = d_t.tile([128, C], bf16, tag='rsx')
                            nc.sync.dma_start(
                                out=rsx[:],
                                in_=rs_out[tt * 128:(tt + 1) * 128, :])
                            xht = d_t.tile([128, C], f32, tag='xht')
                            nc.sync.dma_start(
                                out=xht[:],
                                in_=xh_d[tt * 128:(tt + 1) * 128, :])
                            nc.vector.tensor_add(out=x2_sb[:, tt - 4, :],
                                                 in0=rsx[:], in1=xht[:])
                            x2bt = d_t.tile([128, C], bf16, tag='x2b')
                            nc.vector.tensor_copy(out=x2bt[:],
                                                  in_=x2_sb[:, tt - 4, :])
                            x2b = x2bt[:]
                        ssq2 = d_t.tile([128, 1], f32, tag='ssq2')
                        nc.scalar.activation(out=scr_sq[:],
                                             in_=x2ap(tt),
                                             func=AF.Square, bias=zero_sb[:],
                                             accum_out=ssq2[:])
                        m2 = d_t.tile([128, 1], f32, tag='m2')
                        nc.vector.tensor_scalar(out=m2[:], in0=ssq2[:],
                                                scalar1=float(1.0 / C),
                                                scalar2=EPS,
                                                op0=OP.mult, op1=OP.add)
                        nc.vector.reciprocal_approx_fast(
                            out=rinv_sb[:, tt:tt + 1], in_=m2[:])
                        for jg, (lo, hi) in enumerate(((0, 4), (4, 8),
                                                      (8, 10))):
                            tp2 = dt_ps.tile([128, 512], bf16, tag='tp2')
                            for j in range(lo, hi):
                                nc.tensor.transpose(
                                    tp2[:, (j - lo) * 128:(j - lo + 1) * 128],
                                    x2b[:, j * 128:(j + 1) * 128],
                                    identb[:])
                            nc.vector.tensor_copy(
                                out=xn2T[:, lo:hi, tt * 128:(tt + 1) * 128],
                                in_=tp2[:, 0:(hi - lo) * 128].rearrange(
                                    'p (j d) -> p j d', d=128))

                    wf1_pre = {}

                    def wf1_fetch(fi):
                        wf1 = wf1_pool.tile([128, 10, 128], bf16,
                                            tag='wf1')
                        nc.sync.dma_start(
                            out=wf1[:],
                            in_=wfc1_d[:, fi * 128:(fi + 1) * 128]
                            .rearrange('(j p) c -> p j c', p=128))
                        return wf1

                    def fc1_half(half):
                        lo_t = half * 512
                        for fi in range(F // 128):
                            wf1 = wf1_pre.pop(fi, None) if half == 0 \
                                else None
                            if wf1 is None:
                                wf1 = wf1_fetch(fi)
                            hp = d_ps.tile([128, 512], f32, tag='hps')
                            for j in range(10):
                                nc.tensor.matmul(
                                    hp[:], wf1[:, j, :],
                                    xn2T[:, j, lo_t:lo_t + 512],
                                    start=(j == 0), stop=(j == 9))
                            hrelu = d_t.tile([128, 512], bf16, tag='hrelu')
                            nc.scalar.activation(out=hrelu[:], in_=hp[:],
                                                 func=AF.Relu,
                                                 bias=zero_sb[:])
                            nc.vector.tensor_mul(
                                out=h2T[:, fi, lo_t:lo_t + 512],
                                in0=hrelu[:], in1=hrelu[:])

                    for fi in range(4):
                        wf1_pre[fi] = wf1_fetch(fi)
                    for tt in range(4):
                        mlp_head(tt)
                    fc1_half(0)
                    for tt in range(4, 8):
                        mlp_head(tt)
                    fc1_half(1)

                # fc2: c-halves x tt-quads; 2 matmuls (640 cols) per lhsT
                with tc.tile_pool(name='y_ps', bufs=4, space='PSUM') as y_ps, \
                     tc.tile_pool(name='wf2', bufs=3) as wf2_pool:
                    for clo, chi in ((0, 640), (640, C)):
                        for ttg in range(2):
                            yps = [y_ps.tile([128, 640], f32, tag='yps',
                                             name='yps')
                                   for _ in range(4)]
                            for f2 in range(F // 256):
                                wf2 = wf2_pool.tile([128, 2, 640], bf16,
                                                    tag='wf2')
                                nc.sync.dma_start(
                                    out=wf2[:],
                                    in_=wfc2_d[f2 * 256:(f2 + 1) * 256,
                                               clo:chi]
                                    .rearrange('(u p) c -> p u c', p=128))
                                for u in range(2):
                                    fi = 2 * f2 + u
                                    st = (fi == 0)
                                    sp_ = (fi == F // 128 - 1)
                                    for i in range(4):
                                        tt = 4 * ttg + i
                                        lhsT = h2T[:, fi,
                                                   tt * 128:(tt + 1) * 128]
                                        nc.tensor.matmul(
                                            yps[i][:, 0:512], lhsT,
                                            wf2[:, u, 0:512],
                                            start=st, stop=sp_)
                                        nc.tensor.matmul(
                                            yps[i][:, 512:640], lhsT,
                                            wf2[:, u, 512:640],
                                            start=st, stop=sp_)
                            for i in range(4):
                                tt = 4 * ttg + i
                                yo = d_t.tile([128, 640], f32, tag='yo')
                                nc.vector.scalar_tensor_tensor(
                                    out=yo[:], in0=yps[i][:],
                                    scalar=rinv_sb[:, tt:tt + 1],
                                    in1=x2ap(tt)[:, clo:chi],
                                    op0=OP.mult, op1=OP.add)
                                nc.sync.dma_start(
                                    out=y_d[tt * 128:(tt + 1) * 128,
                                            clo:chi],
                                    in_=yo[:])

    nc.compile()
    return nc


_CACHE = {}


def _get_nc(t_len=T):
    if t_len not in _CACHE:
        _CACHE[t_len] = build_nc(t_len)
    return _CACHE[t_len]


def make_in_maps(x, rotary_pos_emb, ln1_w, w_qkv, qn_w, kn_w, w_out, ln2_w,
                 w_fc1, w_fc2, t_len=T):
    """Host-side sharding prep. Returns list of per-core input dicts."""
    x = np.asarray(x, np.float32)
    rot = np.asarray(rotary_pos_emb, np.float32)
    cos = np.cos(rot).astype(np.float32)
    sin = np.sin(rot).astype(np.float32)
    sinneg = np.concatenate([-sin[:, :64], sin[:, :64]], axis=-1)
    qn = np.asarray(qn_w, np.float32)
    kn = np.asarray(kn_w, np.float32)
    cossin = np.stack([cos * qn, sinneg * qn, cos * kn, sinneg * kn],
                      axis=1).reshape(T, 4 * D).astype(ml_dtypes.bfloat16)
    cossin = np.ascontiguousarray(cossin)
    w_qkv_f = (np.asarray(w_qkv, np.float32) * 1024.0
               * np.asarray(ln1_w, np.float32)[:, None]).reshape(C, 3, H, D)
    w_fc1_f = (np.asarray(w_fc1, np.float32)
               * np.asarray(ln2_w, np.float32)[:, None]
               ).astype(ml_dtypes.bfloat16)
    w_fc2_b = np.asarray(w_fc2, np.float32).astype(ml_dtypes.bfloat16)
    # attn carries a *32 scale; w_out carries *1024 fp8 scale (undone by
    # the 2^-15 psum copy scale on-core)
    wo = (np.asarray(w_out, np.float32) * 32768.0 / QSCALE).reshape(H, D, C)

    # tokens owned per rank within a pair: rank0 tiles {0-3, 8-11},
    # rank1 tiles {4-7, 12-15} (tile = 128 tokens)
    halves = [np.r_[0:512, 1024:1536], np.r_[512:1024, 1536:2048]]

    in_maps = []
    for c in range(N_CORES):
        b, hg = c // 2, c % 2
        heads = slice(hg * HPC, (hg + 1) * HPC)
        wq = np.ascontiguousarray(
            w_qkv_f[:, :, heads, :].reshape(C, 3 * CPC)
        ).astype(ml_dtypes.float8_e4m3)
        w_outp = np.ascontiguousarray(
            wo[heads].reshape(CPC, C)).astype(ml_dtypes.float8_e4m3)
        xb = x[b]
        in_maps.append({
            'xsq': np.ascontiguousarray(xb).astype(ml_dtypes.bfloat16),
            'xT': np.ascontiguousarray(
                xb.T * QSCALE).astype(ml_dtypes.float8_e4m3),
            'xh': np.ascontiguousarray(xb[halves[hg]]),
            'w_qkv': wq,
            'cossin': cossin,
            'w_out': w_outp,
            'w_fc1': np.ascontiguousarray(w_fc1_f),
            'w_fc2': np.ascontiguousarray(w_fc2_b),
        })
    return in_maps


def assemble_output(results, t_len=T):
    halves = [np.r_[0:512, 1024:1536], np.r_[512:1024, 1536:2048]]
    out = np.zeros((B, t_len, C), np.float32)
    for c in range(N_CORES):
        b, hg = c // 2, c % 2
        out[b, halves[hg]] = results[c]['y']
    return out


def kernel(**inputs):
    nc = _get_nc(T)
    in_maps = make_in_maps(**inputs)
    res = bass_utils.run_bass_kernel_spmd(nc, in_maps,
                                          core_ids=list(range(N_CORES)))
    return assemble_output(res.results)


# revision 34
# speedup vs baseline: 1.0108x; 1.0108x over previous
"""Trainium2 Bass kernel for nn_Block_27848567948000 (dense transformer block).

Sharding (8 NeuronCores): 4 data-parallel groups over batch (B=4), 2-way
tensor-parallel within each pair: attention sharded over heads (5 each).
out_proj computed as per-head partial sums over ALL T, summed + token-scattered
via pairwise ReduceScatters; MLP over the core's T/2 tokens.

Token ownership (per pair): rank0 owns tiles {0-3, 8-11}, rank1 owns
{4-7, 12-15}, so ReduceScatter #1 (rows 0:1024 = tiles 0-7) can fire right
after query-block qb=1 and RS#2 after qb=3; fc1 on the first half overlaps
RS#2.

Quantization: the reference's mxfp8 QDQ equals a plain e4m3 cast under a
global power-of-2 scale for all values in fp8-normal range (validated
numerically: rel err ~2e-3 incl. fp8 softmax probs).  q/k are quantized with
scale 32*rstd (rms fold), v with 32; w_out pre-scaled by 1/32 on the host.
NOTE dt.float8e4 is IEEE e4m3: max finite 240, inf above -- scale 32 keeps
|q*32|<=170, |v*32|<=196, and exp bias -1 keeps p<=80.  Softmax probs are fp8,
enabling DoubleRow (2x fp8) matmuls for P@V + denominator over kt-tile pairs.

kernel(**inputs) takes FULL inputs and returns the FULL (4, 2048, 1280) output.
"""
import sys

sys.path.insert(0, '/opt/trn_rl_repo')

import numpy as np
import ml_dtypes

import concourse.bass as bass
import concourse.tile as tile
from concourse import mybir, bacc
from concourse import bass_utils
from concourse.masks import make_identity

B, T, C, H, D, F = 4, 2048, 1280, 10, 128, 5120
EPS = 1e-5
N_CORES = 8
HPC = H // 2            # heads per core (5)
CPC = HPC * D           # channels per core (640)
f32 = mybir.dt.float32
bf16 = mybir.dt.bfloat16
fp8 = mybir.dt.float8e4
i32 = mybir.dt.int32
AF = mybir.ActivationFunctionType
OP = mybir.AluOpType
AX = mybir.AxisListType
DR = mybir.MatmulPerfMode.DoubleRow

NT = T // 128            # 16 token tiles
NH = T // 2 // 128       # 8 token tiles in my half
QB = T // 512            # 4 query blocks
INV_SQRT_D = float(1.0 / np.sqrt(D))
EXP_BIAS = -1.0
QSCALE = 32.0   # global fp8 scale for q/k/v (e4m3 max finite is 240!)


def _rsqrt_vec(nc, pool, out_ap, in_ap, scale, eps, tag, eng=None):
    """out = 1/sqrt(in*scale + eps) on a DVE-like engine (no act tables).
    Bit-trick seed + 2 Newton iterations (~1e-6 rel err). Shapes (128, n)."""
    if eng is None:
        eng = nc.vector
    i32_ = mybir.dt.int32
    shp = [128, in_ap.free_size()]
    m = pool.tile(shp, f32, tag=tag + 'm', name='rs_m')
    eng.tensor_scalar(out=m[:], in0=in_ap, scalar1=scale, scalar2=eps,
                      op0=OP.mult, op1=OP.add)
    y = pool.tile(shp, f32, tag=tag + 'y', name='rs_y')
    eng.tensor_single_scalar(out=y[:].bitcast(i32_),
                             in_=m[:].bitcast(i32_), scalar=1,
                             op=OP.logical_shift_right)
    eng.tensor_scalar(out=y[:].bitcast(i32_), in0=y[:].bitcast(i32_),
                      scalar1=-1, scalar2=0x5f3759df,
                      op0=OP.mult, op1=OP.add)
    t = pool.tile(shp, f32, tag=tag + 't', name='rs_t')
    for it in range(2):
        eng.tensor_tensor(out=t[:], in0=y[:], in1=y[:], op=OP.mult)
        eng.tensor_tensor(out=t[:], in0=t[:], in1=m[:], op=OP.mult)
        eng.tensor_scalar(out=t[:], in0=t[:], scalar1=-0.5,
                          scalar2=1.5, op0=OP.mult, op1=OP.add)
        eng.tensor_tensor(out=y[:] if it == 0 else out_ap, in0=y[:],
                          in1=t[:], op=OP.mult)


def _ap(t_ap, offset_delta, pattern):
    return bass.AP(tensor=t_ap.tensor, offset=t_ap.offset + offset_delta,
                   ap=pattern)


def build_nc(t_len=T, n_cores=N_CORES):
    import contextlib
    nc = bacc.Bacc('TRN2', target_bir_lowering=False, debug=False,
                   num_devices=n_cores)

    # ---- DRAM I/O ----
    # xsq: row-major bf16 x (for rmsnorm sum-of-squares)
    # xT:  transposed bf16 x [C, T] (QKV lhsT; no PE transposes needed)
    xsq_d = nc.dram_tensor('xsq', [T, C], bf16, kind='ExternalInput')
    xT_d = nc.dram_tensor('xT', [C, T], fp8, kind='ExternalInput')
    xh_d = nc.dram_tensor('xh', [T // 2, C], f32, kind='ExternalInput')
    wqkv_d = nc.dram_tensor('w_qkv', [C, 3 * CPC], fp8, kind='ExternalInput')
    cossin_d = nc.dram_tensor('cossin', [T, 4 * D], bf16,
                              kind='ExternalInput')
    wout_d = nc.dram_tensor('w_out', [CPC, C], fp8, kind='ExternalInput')
    wfc1_d = nc.dram_tensor('w_fc1', [C, F], bf16, kind='ExternalInput')
    wfc2_d = nc.dram_tensor('w_fc2', [F, C], bf16, kind='ExternalInput')
    y_d = nc.dram_tensor('y', [T // 2, C], f32, kind='ExternalOutput')

    with tile.TileContext(nc) as tc:
        with contextlib.ExitStack() as ctx:
            persist = ctx.enter_context(tc.tile_pool(name='persist', bufs=1))
            dram = ctx.enter_context(tc.tile_pool(name='dram', bufs=1,
                                                  space='DRAM'))

            # ---- constants ----
            identb = persist.tile([128, 128], bf16)
            make_identity(nc, identb)
            ones256 = persist.tile([128, 256], fp8)
            nc.vector.memset(ones256[:], 1.0)
            zero_sb = persist.tile([128, 1], f32)
            nc.vector.memset(zero_sb[:], 0.0)
            ebias_sb = persist.tile([128, 1], f32)
            nc.vector.memset(ebias_sb[:], EXP_BIAS)
            scr_sq = persist.tile([128, C], bf16)   # Square-output scratch
            # residual x2 for my first 4 tiles, staged during qb3 so phase D
            # can start fc1 immediately (overlapping RS#2)
            x2a = persist.tile([128, 4, C], f32)
            x2b4 = persist.tile([128, 4, C], bf16)

            # DRAM scratch for the collectives -- separate tiles per chunk
            # so oproj writes for chunk B never carry a false WAR dependency
            # on RS#1's read of chunk A
            rs_in_a = dram.tile([T // 2, C], bf16)
            rs_in_b = dram.tile([T // 2, C], bf16)
            rs_out_a = dram.tile([T // 4, C], bf16)
            rs_out_b = dram.tile([T // 4, C], bf16)

            with contextlib.ExitStack() as pab:
                ab = pab.enter_context(tc.tile_pool(name='ab', bufs=1))
                qT = ab.tile([128, HPC, T], bf16)
                kT = ab.tile([128, HPC, T], bf16)
                vd_sb = ab.tile([128, NT, HPC, D], fp8)
                at_p = pab.enter_context(tc.tile_pool(name='at_p', bufs=2))

                # ====== phases A+B ======
                with contextlib.ExitStack() as pin:
                    a_w = pin.enter_context(tc.tile_pool(name='a_w', bufs=1))
                    a_x = pin.enter_context(tc.tile_pool(name='a_x', bufs=3))
                    wq_sb = a_w.tile([128, 10, 3 * CPC], fp8)
                    cs_sb = a_w.tile([128, NT, 4, D], bf16)

                    def load_aw():
                        nc.sync.dma_start(
                            out=wq_sb[:],
                            in_=wqkv_d.ap().rearrange('(j p) c -> p j c',
                                                      p=128))
                        nc.sync.dma_start(
                            out=cs_sb[:],
                            in_=cossin_d.ap().rearrange('(t p) x -> p t x',
                                                        p=128))

                    a_t = pin.enter_context(tc.tile_pool(name='a_t', bufs=2))
                    a_s = pin.enter_context(tc.tile_pool(name='a_s', bufs=2))
                    a_q = pin.enter_context(tc.tile_pool(name='a_q', bufs=4))
                    pT_pool = pin.enter_context(
                        tc.tile_pool(name='pT', bufs=4))
                    b_t = pin.enter_context(tc.tile_pool(name='b_t', bufs=2))
                    ps512 = pin.enter_context(
                        tc.tile_pool(name='ps512', bufs=3, space='PSUM'))
                    ops_ps = pin.enter_context(
                        tc.tile_pool(name='ops_ps', bufs=2, space='PSUM'))
                    psT = pin.enter_context(
                        tc.tile_pool(name='psT', bufs=2, space='PSUM'))
                    psD = pin.enter_context(
                        tc.tile_pool(name='psD', bufs=1, space='PSUM'))

                    stash = {}

                    def emit_head(t, first=False):
                        # rstd of x for this token tile (from bf16 x rows)
                        xt = a_s.tile([128, C], bf16, tag='xt')
                        nc.sync.dma_start(
                            out=xt[:], in_=xsq_d[t * 128:(t + 1) * 128, :])
                        xTt = a_x.tile([128, 10, 128], fp8, tag='xTt')
                        nc.sync.dma_start(
                            out=xTt[:],
                            in_=xT_d[:, t * 128:(t + 1) * 128]
                            .rearrange('(j p) t -> p j t', p=128))
                        if first:
                            load_aw()
                        ssq = a_s.tile([128, 1], f32, tag='ssq')
                        nc.scalar.activation(out=scr_sq[:], in_=xt[:],
                                             func=AF.Square, bias=zero_sb[:],
                                             accum_out=ssq[:])
                        # rstd scaled by 2^-15 to undo x*32 and w*1024 fp8
                        # scaling: rsqrt((ssq/C + EPS) * 2^30)
                        rstd = a_s.tile([128, 1], f32, tag='rstd')
                        _rsqrt_vec(nc, a_s, rstd[:], ssq[:],
                                   float((2.0 ** 30) / C),
                                   float(EPS * (2.0 ** 30)), 'rx')
                        # QKV (chunk-outer, j-mid, g-inner: LDW amortized)
                        qf = a_q.tile([128, CPC], bf16, tag='qf')
                        kf = a_q.tile([128, CPC], bf16, tag='kf')
                        vf = a_q.tile([128, CPC], bf16, tag='vf')
                        dsts = (qf, kf, vf)
                        for lo, hi in ((0, 512), (512, 640)):
                            pss = [ps512.tile([128, 512], f32, tag='mm',
                                              name='qkvps')
                                   for _ in range(3)]
                            for jp in range(5):
                                for g in range(3):
                                    nc.tensor.matmul(
                                        pss[g][:, 0:hi - lo],
                                        xTt[:, 2 * jp:2 * jp + 2, :],
                                        wq_sb[:, 2 * jp:2 * jp + 2,
                                              g * CPC + lo:g * CPC + hi],
                                        start=(jp == 0), stop=(jp == 4),
                                        perf_mode=DR)
                            for g in range(3):
                                if g == 0:
                                    nc.vector.tensor_scalar_mul(
                                        out=dsts[g][:, lo:hi],
                                        in0=pss[g][:, 0:hi - lo],
                                        scalar1=rstd[:])
                                else:
                                    nc.scalar.activation(
                                        out=dsts[g][:, lo:hi],
                                        in_=pss[g][:, 0:hi - lo],
                                        func=AF.Copy, scale=rstd[:])
                        stash[t] = (qf, kf, vf)

                    def rope(eng, src, cos_t, sin_t, out):
                        # out[p,h,d] = src*cos + swap(src)*sinneg   (bf16)
                        src3 = src[:].rearrange('p (h d) -> p h d', h=HPC)
                        pa = list(src3.ap)
                        swap = _ap(src3, 64, pa[:2] + [[-64, 2], [1, 64]])
                        ca = list(cos_t.ap)
                        cos4 = _ap(cos_t, 0, [ca[0], [0, HPC], [1, 128]])
                        sin4 = _ap(sin_t, 0,
                                   [ca[0], [0, HPC], [64, 2], [1, 64]])
                        tmp = a_t.tile([128, HPC, D], bf16, tag='rtmp')
                        eng.tensor_tensor(
                            out=tmp[:].rearrange('p h (u d) -> p h u d', u=2),
                            in0=swap, in1=sin4, op=OP.mult)
                        eng.tensor_tensor(out=out[:], in0=src3, in1=cos4,
                                          op=OP.mult)
                        eng.tensor_add(out=out[:], in0=out[:], in1=tmp[:])

                    def hb(ap5, reps):
                        # (128,5) -> (128,5,reps) broadcast
                        a = list(ap5.ap)
                        return bass.AP(tensor=ap5.tensor, offset=ap5.offset,
                                       ap=[a[0], [a[-1][0], HPC], [0, reps]])

                    def emit_tail(t):
                        qf, kf, vf = stash.pop(t)
                        # rms of pre-rope q/k (rope is norm-preserving);
                        # msq64 = 64 * rstd (the fp8 global scale folded in)
                        msq = a_t.tile([128, 2, HPC], f32, tag='msq')
                        qsq = a_t.tile([128, HPC, D], bf16, tag='qsq')
                        qf3 = qf[:].rearrange('p (h d) -> p h d', h=HPC)
                        nc.vector.tensor_tensor(out=qsq[:], in0=qf3,
                                                in1=qf3, op=OP.mult)
                        nc.vector.tensor_reduce(out=msq[:, 0, :],
                                                in_=qsq[:], axis=AX.X,
                                                op=OP.add)
                        ksq = a_t.tile([128, HPC, D], bf16, tag='ksq')
                        kf3 = kf[:].rearrange('p (h d) -> p h d', h=HPC)
                        nc.vector.tensor_tensor(out=ksq[:], in0=kf3,
                                                in1=kf3, op=OP.mult)
                        nc.vector.tensor_reduce(out=msq[:, 1, :],
                                                in_=ksq[:], axis=AX.X,
                                                op=OP.add)
                        _rsqrt_vec(nc, a_t, msq[:], msq[:],
                                   float(1.0 / (D * QSCALE * QSCALE)),
                                   float(EPS / (QSCALE * QSCALE)), 'rqk')
                        # rope (q on vector, k on gpsimd)
                        zq = a_t.tile([128, HPC, D], bf16, tag='zq')
                        rope(nc.vector, qf, cs_sb[:, t, 0, :],
                             cs_sb[:, t, 1, :], zq)
                        zk = a_t.tile([128, HPC, D], bf16, tag='zk')
                        rope(nc.vector, kf, cs_sb[:, t, 2, :],
                             cs_sb[:, t, 3, :], zk)
                        # quantize: one op per tensor (global-scale e4m3)
                        q8v = a_t.tile([128, HPC, D], fp8, tag='q8')
                        nc.vector.tensor_tensor(out=q8v[:], in0=zq[:],
                                                in1=hb(msq[:, 0, :], D),
                                                op=OP.mult)
                        k8v = a_t.tile([128, HPC, D], fp8, tag='k8')
                        nc.vector.tensor_tensor(out=k8v[:], in0=zk[:],
                                                in1=hb(msq[:, 1, :], D),
                                                op=OP.mult)
                        nc.vector.tensor_scalar_mul(
                            out=vd_sb[:, t, :, :],
                            in0=vf[:].rearrange('p (h d) -> p h d', h=HPC),
                            scalar1=QSCALE)
                        # cast quantized q/k back to bf16 (exact) for the
                        # PE transposes (walrus rejects fp8 transposes)
                        qdb = a_t.tile([128, HPC, D], bf16, tag='qdb')
                        nc.scalar.copy(out=qdb[:], in_=q8v[:])
                        kdb = a_t.tile([128, HPC, D], bf16, tag='kdb')
                        nc.scalar.copy(out=kdb[:], in_=k8v[:])
                        for src, dstT in ((qdb, qT), (kdb, kT)):
                            tp = psT.tile([128, 640], bf16, tag='tp')
                            for h in range(HPC):
                                nc.tensor.transpose(
                                    tp[:, h * 128:(h + 1) * 128],
                                    src[:, h, :], identb[:])
                            nc.vector.tensor_copy(
                                out=dstT[:, :, t * 128:(t + 1) * 128],
                                in_=tp[:].rearrange('p (h d) -> p h d',
                                                    h=HPC))

                    def emit_attn_h(qb, h, at):
                        nkt = 4 * qb + 4
                        dps = psD.tile([128, 512], f32, tag='dps')
                        ops = ops_ps.tile([128, 512], f32, tag='ops')
                        for kp in range(nkt // 2):
                            pT2 = pT_pool.tile([128, 2, 512], fp8, tag='pT')
                            for u in range(2):
                                kt = 2 * kp + u
                                sp = ps512.tile([128, 512], f32, tag='mm')
                                nc.tensor.matmul(
                                    sp[:],
                                    kT[:, h, kt * 128:(kt + 1) * 128],
                                    qT[:, h, qb * 512:(qb + 1) * 512],
                                    start=True, stop=True)
                                nc.scalar.activation(
                                    out=pT2[:, u, :], in_=sp[:],
                                    func=AF.Exp, bias=ebias_sb[:],
                                    scale=float(INV_SQRT_D /
                                                (QSCALE * QSCALE)))
                                o = kt - 4 * qb
                                if o >= 0:
                                    nc.gpsimd.affine_select(
                                        out=pT2[:, u, :], in_=pT2[:, u, :],
                                        compare_op=OP.is_ge, fill=0.0,
                                        base=-128 * o, pattern=[[1, 512]],
                                        channel_multiplier=-1)
                            st = (kp == 0)
                            sp_ = (kp == nkt // 2 - 1)
                            nc.tensor.matmul(
                                dps[:],
                                ones256[:].rearrange('p (u m) -> p u m', u=2),
                                pT2[:], start=st, stop=sp_, perf_mode=DR)
                            nc.tensor.matmul(
                                ops[:], vd_sb[:, 2 * kp:2 * kp + 2, h, :],
                                pT2[:], start=st, stop=sp_, perf_mode=DR)
                        rd = b_t.tile([128, 512], f32, tag='rd')
                        nc.vector.reciprocal_approx_fast(out=rd[:],
                                                         in_=dps[:])
                        nc.vector.tensor_tensor(
                            out=at[:, h, :],
                            in0=ops[:], in1=rd[:], op=OP.mult)

                    wo_sb = a_w.tile([128, HPC, C], fp8)
                    nc.sync.dma_start(
                        out=wo_sb[:],
                        in_=wout_d.ap().rearrange('(h p) c -> p h c', p=128))
                    grp = [[2 * i, 2 * i + 1] for i in range(n_cores // 2)]

                    OSC = float(2.0 ** -15)   # undo attn*32 and wo*1024

                    def oproj(tt, at):
                        tl = (tt % 4) * 128
                        ob = b_t.tile([128, C], bf16, tag='ob', name='ob')
                        for ci, (lo, hi) in enumerate(((0, 512),
                                                       (512, 1024),
                                                       (1024, C))):
                            ps = ps512.tile([128, 512], f32, tag='mm',
                                            name='oprojps')
                            for hp2 in range(2):
                                nc.tensor.matmul(
                                    ps[:, 0:hi - lo],
                                    at[:, 2 * hp2:2 * hp2 + 2, tl:tl + 128],
                                    wo_sb[:, 2 * hp2:2 * hp2 + 2, lo:hi],
                                    start=(hp2 == 0), stop=False,
                                    perf_mode=DR)
                            nc.tensor.matmul(
                                ps[:, 0:hi - lo], at[:, 4, tl:tl + 128],
                                wo_sb[:, 4, lo:hi],
                                start=False, stop=True)
                            if ci == 2:
                                nc.scalar.activation(
                                    out=ob[:, lo:hi],
                                    in_=ps[:, 0:hi - lo],
                                    func=AF.Copy, scale=OSC)
                            else:
                                nc.vector.tensor_scalar_mul(
                                    out=ob[:, lo:hi],
                                    in0=ps[:, 0:hi - lo], scalar1=OSC)
                        rs_dst = rs_in_a if tt < 8 else rs_in_b
                        r = tt if tt < 8 else tt - 8
                        nc.sync.dma_start(
                            out=rs_dst[r * 128:(r + 1) * 128, :], in_=ob[:])

                    # ---- interleaved A+B+C emission ----
                    # attn/oproj units of query-block qb are dripped across
                    # the next 4 head/tail slots so the PE FIFO always has
                    # QKV work queued behind exp-gated score tiles.
                    # RS#1 (tiles 0-7) fires after qb=1's oproj; RS#2 at end.
                    pending = []

                    def drip(n):
                        for _ in range(n):
                            if pending:
                                pending.pop(0)()

                    def mk_attn(qb, h, at):
                        return lambda: emit_attn_h(qb, h, at)

                    def mk_oproj(tt, at):
                        return lambda: oproj(tt, at)

                    def mk_rs1():
                        def f():
                            nc.gpsimd.collective_compute(
                                'ReduceScatter', OP.add,
                                ins=[rs_in_a[0:1024, :].opt()],
                                outs=[rs_out_a[0:512, :].opt()],
                                replica_groups=grp)
                        return f

                    for t in range(NT):
                        emit_head(t, first=(t == 0))
                        if t >= 1:
                            emit_tail(t - 1)
                        drip(3)
                        if t >= 4 and t % 4 == 0:
                            qb = t // 4 - 1
                            at = at_p.tile([128, HPC, 512], fp8, tag='at')
                            for h in range(HPC):
                                pending.append(mk_attn(qb, h, at))
                            if qb == 2:
                                # RS#1 trigger goes on the gpsimd queue; it
                                # must come after qb2's affine_selects there
                                # or they stall behind the CC completion.
                                pending.append(mk_rs1())
                            for tt in range(4 * qb, 4 * qb + 4):
                                pending.append(mk_oproj(tt, at))
                    def mlp_pre(tt):
                        # x2 = rs_out + xh for my tiles 0-3 (RS#1 data)
                        rsx = b_t.tile([128, C], bf16, tag='rsx')
                        nc.sync.dma_start(
                            out=rsx[:],
                            in_=rs_out_a[tt * 128:(tt + 1) * 128, :])
                        xht = b_t.tile([128, C], f32, tag='xht')
                        nc.sync.dma_start(
                            out=xht[:],
                            in_=xh_d[tt * 128:(tt + 1) * 128, :])
                        nc.vector.tensor_add(out=x2a[:, tt, :],
                                             in0=rsx[:], in1=xht[:])
                        nc.vector.tensor_copy(out=x2b4[:, tt, :],
                                              in_=x2a[:, tt, :])

                    emit_tail(NT - 1)
                    drip(len(pending))
                    at = at_p.tile([128, HPC, 512], fp8, tag='at')
                    for h in range(HPC):
                        emit_attn_h(QB - 1, h, at)
                        if h < 4:
                            mlp_pre(h)
                    for tt in range(12, 16):
                        oproj(tt, at)
                    nc.gpsimd.collective_compute(
                        'ReduceScatter', OP.add,
                        ins=[rs_in_b[0:1024, :].opt()],
                        outs=[rs_out_b[0:512, :].opt()],
                        replica_groups=grp)

            # ====== phase D: residual + MLP over my T/2 tokens ======
            # half 0 (tiles 0-3) depends only on RS#1 -> overlaps RS#2.
            with contextlib.ExitStack() as pd:
                d_t = pd.enter_context(tc.tile_pool(name='d_t', bufs=2))
                d_big = pd.enter_context(tc.tile_pool(name='d_big', bufs=1))
                x2_sb = d_big.tile([128, 4, C], f32)
                xn2T = d_big.tile([128, 10, T // 2], bf16)
                h2T = d_big.tile([128, F // 128, T // 2], bf16)
                rinv_sb = d_big.tile([128, NH], f32)

                with tc.tile_pool(name='d_ps', bufs=4, space='PSUM') as d_ps, \
                     tc.tile_pool(name='dt_ps', bufs=2,
                                  space='PSUM') as dt_ps, \
                     tc.tile_pool(name='wf1', bufs=5) as wf1_pool:

                    def x2ap(tt):
                        return x2a[:, tt, :] if tt < 4 \
                            else x2_sb[:, tt - 4, :]

                    def mlp_head(tt):
                        # x2 = rs_out + xh; rstd; transposes into xn2T
                        if tt < 4:
                            x2b = x2b4[:, tt, :]
                        else:
                            rsx = d_t.tile([128, C], bf16, tag='rsx')
                            nc.sync.dma_start(
                                out=rsx[:],
                                in_=rs_out_b[(tt - 4) * 128:(tt - 3) * 128,
                                             :])
                            xht = d_t.tile([128, C], f32, tag='xht')
                            nc.sync.dma_start(
                                out=xht[:],
                                in_=xh_d[tt * 128:(tt + 1) * 128, :])
                            nc.vector.tensor_add(out=x2_sb[:, tt - 4, :],
                                                 in0=rsx[:], in1=xht[:])
                            x2bt = d_t.tile([128, C], bf16, tag='x2b')
                            nc.vector.tensor_copy(out=x2bt[:],
                                                  in_=x2_sb[:, tt - 4, :])
                            x2b = x2bt[:]
                        ssq2 = d_t.tile([128, 1], f32, tag='ssq2')
                        nc.scalar.activation(out=scr_sq[:],
                                             in_=x2ap(tt),
                                             func=AF.Square, bias=zero_sb[:],
                                             accum_out=ssq2[:])
                        m2 = d_t.tile([128, 1], f32, tag='m2')
                        nc.vector.tensor_scalar(out=m2[:], in0=ssq2[:],
                                                scalar1=float(1.0 / C),
                                                scalar2=EPS,
                                                op0=OP.mult, op1=OP.add)
                        nc.vector.reciprocal_approx_fast(
                            out=rinv_sb[:, tt:tt + 1], in_=m2[:])
                        for jg, (lo, hi) in enumerate(((0, 4), (4, 8),
                                                      (8, 10))):
                            tp2 = dt_ps.tile([128, 512], bf16, tag='tp2')
                            for j in range(lo, hi):
                                nc.tensor.transpose(
                                    tp2[:, (j - lo) * 128:(j - lo + 1) * 128],
                                    x2b[:, j * 128:(j + 1) * 128],
                                    identb[:])
                            nc.vector.tensor_copy(
                                out=xn2T[:, lo:hi, tt * 128:(tt + 1) * 128],
                                in_=tp2[:, 0:(hi - lo) * 128].rearrange(
                                    'p (j d) -> p j d', d=128))

                    wf1_pre = {}

                    def wf1_fetch(fi):
                        wf1 = wf1_pool.tile([128, 10, 128], bf16,
                                            tag='wf1')
                        nc.sync.dma_start(
                            out=wf1[:],
                            in_=wfc1_d[:, fi * 128:(fi + 1) * 128]
                            .rearrange('(j p) c -> p j c', p=128))
                        return wf1

                    def fc1_half(half):
                        lo_t = half * 512
                        for fi in range(F // 128):
                            wf1 = wf1_pre.pop(fi, None) if half == 0 \
                                else None
                            if wf1 is None:
                                wf1 = wf1_fetch(fi)
                            hp = d_ps.tile([128, 512], f32, tag='hps')
                            for j in range(10):
                                nc.tensor.matmul(
                                    hp[:], wf1[:, j, :],
                                    xn2T[:, j, lo_t:lo_t + 512],
                                    start=(j == 0), stop=(j == 9))
                            hrelu = d_t.tile([128, 512], bf16, tag='hrelu')
                            nc.scalar.activation(out=hrelu[:], in_=hp[:],
                                                 func=AF.Relu,
                                                 bias=zero_sb[:])
                            nc.vector.tensor_mul(
                                out=h2T[:, fi, lo_t:lo_t + 512],
                                in0=hrelu[:], in1=hrelu[:])

                    for fi in range(4):
                        wf1_pre[fi] = wf1_fetch(fi)
                    for tt in range(4):
                        mlp_head(tt)
                    fc1_half(0)
                    for tt in range(4, 8):
                        mlp_head(tt)
                    fc1_half(1)

                # fc2: c-halves x tt-quads; 2 matmuls (640 cols) per lhsT
                with tc.tile_pool(name='y_ps', bufs=4, space='PSUM') as y_ps, \
                     tc.tile_pool(name='wf2', bufs=3) as wf2_pool:
                    for clo, chi in ((0, 640), (640, C)):
                        for ttg in range(2):
                            yps = [y_ps.tile([128, 640], f32, tag='yps',
                                             name='yps')
                                   for _ in range(4)]
                            for f2 in range(F // 256):
                                wf2 = wf2_pool.tile([128, 2, 640], bf16,
                                                    tag='wf2')
                                nc.sync.dma_start(
                                    out=wf2[:],
                                    in_=wfc2_d[f2 * 256:(f2 + 1) * 256,
                                               clo:chi]
                                    .rearrange('(u p) c -> p u c', p=128))
                                for u in range(2):
                                    fi = 2 * f2 + u
                                    st = (fi == 0)
                                    sp_ = (fi == F // 128 - 1)
                                    for i in range(4):
                                        tt = 4 * ttg + i
                                        lhsT = h2T[:, fi,
                                                   tt * 128:(tt + 1) * 128]
                                        nc.tensor.matmul(
                                            yps[i][:, 0:512], lhsT,
                                            wf2[:, u, 0:512],
                                            start=st, stop=sp_)
                                        nc.tensor.matmul(
                                            yps[i][:, 512:640], lhsT,
                                            wf2[:, u, 512:640],
                                            start=st, stop=sp_)
                            for i in range(4):
                                tt = 4 * ttg + i
                                yo = d_t.tile([128, 640], f32, tag='yo')
                                nc.vector.scalar_tensor_tensor(
                                    out=yo[:], in0=yps[i][:],
                                    scalar=rinv_sb[:, tt:tt + 1],
                                    in1=x2ap(tt)[:, clo:chi],
                                    op0=OP.mult, op1=OP.add)
                                nc.sync.dma_start(
                                    out=y_d[tt * 128:(tt + 1) * 128,
                                            clo:chi],
                                    in_=yo[:])

    nc.compile()
    return nc


_CACHE = {}


def _get_nc(t_len=T):
    if t_len not in _CACHE:
        _CACHE[t_len] = build_nc(t_len)
    return _CACHE[t_len]


def make_in_maps(x, rotary_pos_emb, ln1_w, w_qkv, qn_w, kn_w, w_out, ln2_w,
                 w_fc1, w_fc2, t_len=T):
    """Host-side sharding prep. Returns list of per-core input dicts."""
    x = np.asarray(x, np.float32)
    rot = np.asarray(rotary_pos_emb, np.float32)
    cos = np.cos(rot).astype(np.float32)
    sin = np.sin(rot).astype(np.float32)
    sinneg = np.concatenate([-sin[:, :64], sin[:, :64]], axis=-1)
    qn = np.asarray(qn_w, np.float32)
    kn = np.asarray(kn_w, np.float32)
    cossin = np.stack([cos * qn, sinneg * qn, cos * kn, sinneg * kn],
                      axis=1).reshape(T, 4 * D).astype(ml_dtypes.bfloat16)
    cossin = np.ascontiguousarray(cossin)
    w_qkv_f = (np.asarray(w_qkv, np.float32) * 1024.0
               * np.asarray(ln1_w, np.float32)[:, None]).reshape(C, 3, H, D)
    w_fc1_f = (np.asarray(w_fc1, np.float32)
               * np.asarray(ln2_w, np.float32)[:, None]
               ).astype(ml_dtypes.bfloat16)
    w_fc2_b = np.asarray(w_fc2, np.float32).astype(ml_dtypes.bfloat16)
    # attn carries a *32 scale; w_out carries *1024 fp8 scale (undone by
    # the 2^-15 psum copy scale on-core)
    wo = (np.asarray(w_out, np.float32) * 32768.0 / QSCALE).reshape(H, D, C)

    # tokens owned per rank within a pair: rank0 tiles {0-3, 8-11},
    # rank1 tiles {4-7, 12-15} (tile = 128 tokens)
    halves = [np.r_[0:512, 1024:1536], np.r_[512:1024, 1536:2048]]

    in_maps = []
    for c in range(N_CORES):
        b, hg = c // 2, c % 2
        heads = slice(hg * HPC, (hg + 1) * HPC)
        wq = np.ascontiguousarray(
            w_qkv_f[:, :, heads, :].reshape(C, 3 * CPC)
        ).astype(ml_dtypes.float8_e4m3)
        w_outp = np.ascontiguousarray(
            wo[heads].reshape(CPC, C)).astype(ml_dtypes.float8_e4m3)
        xb = x[b]
        in_maps.append({
            'xsq': np.ascontiguousarray(xb).astype(ml_dtypes.bfloat16),
            'xT': np.ascontiguousarray(
                xb.T * QSCALE).astype(ml_dtypes.float8_e4m3),
            'xh': np.ascontiguousarray(xb[halves[hg]]),
            'w_qkv': wq,
            'cossin': cossin,
            'w_out': w_outp,
            'w_fc1': np.ascontiguousarray(w_fc1_f),
            'w_fc2': np.ascontiguousarray(w_fc2_b),
        })
    return in_maps


def assemble_output(results, t_len=T):
    halves = [np.r_[0:512, 1024:1536], np.r_[512:1024, 1536:2048]]
    out = np.zeros((B, t_len, C), np.float32)
    for c in range(N_CORES):
        b, hg = c // 2, c % 2
        out[b, halves[hg]] = results[c]['y']
    return out


def kernel(**inputs):
    nc = _get_nc(T)
    in_maps = make_in_maps(**inputs)
    res = bass_utils.run_bass_kernel_spmd(nc, in_maps,
                                          core_ids=list(range(N_CORES)))
    return assemble_output(res.results)


# revision 35
# speedup vs baseline: 1.0165x; 1.0057x over previous
"""Trainium2 Bass kernel for nn_Block_27848567948000 (dense transformer block).

Sharding (8 NeuronCores): 4 data-parallel groups over batch (B=4), 2-way
tensor-parallel within each pair: attention sharded over heads (5 each).
out_proj computed as per-head partial sums over ALL T, summed + token-scattered
via pairwise ReduceScatters; MLP over the core's T/2 tokens.

Token ownership (per pair): rank0 owns tiles {0-3, 8-11}, rank1 owns
{4-7, 12-15}, so ReduceScatter #1 (rows 0:1024 = tiles 0-7) can fire right
after query-block qb=1 and RS#2 after qb=3; fc1 on the first half overlaps
RS#2.

Quantization: the reference's mxfp8 QDQ equals a plain e4m3 cast under a
global power-of-2 scale for all values in fp8-normal range (validated
numerically: rel err ~2e-3 incl. fp8 softmax probs).  q/k are quantized with
scale 32*rstd (rms fold), v with 32; w_out pre-scaled by 1/32 on the host.
NOTE dt.float8e4 is IEEE e4m3: max finite 240, inf above -- scale 32 keeps
|q*32|<=170, |v*32|<=196, and exp bias -1 keeps p<=80.  Softmax probs are fp8,
enabling DoubleRow (2x fp8) matmuls for P@V + denominator over kt-tile pairs.

kernel(**inputs) takes FULL inputs and returns the FULL (4, 2048, 1280) output.
"""
import sys

sys.path.insert(0, '/opt/trn_rl_repo')

import numpy as np
import ml_dtypes

import concourse.bass as bass
import concourse.tile as tile
from concourse import mybir, bacc
from concourse import bass_utils
from concourse.masks import make_identity

B, T, C, H, D, F = 4, 2048, 1280, 10, 128, 5120
EPS = 1e-5
N_CORES = 8
HPC = H // 2            # heads per core (5)
CPC = HPC * D           # channels per core (640)
f32 = mybir.dt.float32
bf16 = mybir.dt.bfloat16
fp8 = mybir.dt.float8e4
i32 = mybir.dt.int32
AF = mybir.ActivationFunctionType
OP = mybir.AluOpType
AX = mybir.AxisListType
DR = mybir.MatmulPerfMode.DoubleRow

NT = T // 128            # 16 token tiles
NH = T // 2 // 128       # 8 token tiles in my half
QB = T // 512            # 4 query blocks
INV_SQRT_D = float(1.0 / np.sqrt(D))
EXP_BIAS = -1.0
QSCALE = 32.0   # global fp8 scale for q/k/v (e4m3 max finite is 240!)


def _rsqrt_vec(nc, pool, out_ap, in_ap, scale, eps, tag, eng=None):
    """out = 1/sqrt(in*scale + eps) on a DVE-like engine (no act tables).
    Bit-trick seed + 2 Newton iterations (~1e-6 rel err). Shapes (128, n)."""
    if eng is None:
        eng = nc.vector
    i32_ = mybir.dt.int32
    shp = [128, in_ap.free_size()]
    m = pool.tile(shp, f32, tag=tag + 'm', name='rs_m')
    eng.tensor_scalar(out=m[:], in0=in_ap, scalar1=scale, scalar2=eps,
                      op0=OP.mult, op1=OP.add)
    y = pool.tile(shp, f32, tag=tag + 'y', name='rs_y')
    eng.tensor_single_scalar(out=y[:].bitcast(i32_),
                             in_=m[:].bitcast(i32_), scalar=1,
                             op=OP.logical_shift_right)
    eng.tensor_scalar(out=y[:].bitcast(i32_), in0=y[:].bitcast(i32_),
                      scalar1=-1, scalar2=0x5f3759df,
                      op0=OP.mult, op1=OP.add)
    t = pool.tile(shp, f32, tag=tag + 't', name='rs_t')
    for it in range(2):
        eng.tensor_tensor(out=t[:], in0=y[:], in1=y[:], op=OP.mult)
        eng.tensor_tensor(out=t[:], in0=t[:], in1=m[:], op=OP.mult)
        eng.tensor_scalar(out=t[:], in0=t[:], scalar1=-0.5,
                          scalar2=1.5, op0=OP.mult, op1=OP.add)
        eng.tensor_tensor(out=y[:] if it == 0 else out_ap, in0=y[:],
                          in1=t[:], op=OP.mult)


def _ap(t_ap, offset_delta, pattern):
    return bass.AP(tensor=t_ap.tensor, offset=t_ap.offset + offset_delta,
                   ap=pattern)


def build_nc(t_len=T, n_cores=N_CORES):
    import contextlib
    nc = bacc.Bacc('TRN2', target_bir_lowering=False, debug=False,
                   num_devices=n_cores)

    # ---- DRAM I/O ----
    # xsq: row-major bf16 x (for rmsnorm sum-of-squares)
    # xT:  transposed bf16 x [C, T] (QKV lhsT; no PE transposes needed)
    xsq_d = nc.dram_tensor('xsq', [T, C], bf16, kind='ExternalInput')
    xT_d = nc.dram_tensor('xT', [C, T], fp8, kind='ExternalInput')
    xh_d = nc.dram_tensor('xh', [T // 2, C], f32, kind='ExternalInput')
    wqkv_d = nc.dram_tensor('w_qkv', [C, 3 * CPC], fp8, kind='ExternalInput')
    cossin_d = nc.dram_tensor('cossin', [T, 4 * D], bf16,
                              kind='ExternalInput')
    wout_d = nc.dram_tensor('w_out', [CPC, C], fp8, kind='ExternalInput')
    wfc1_d = nc.dram_tensor('w_fc1', [C, F], bf16, kind='ExternalInput')
    wfc2_d = nc.dram_tensor('w_fc2', [F, C], bf16, kind='ExternalInput')
    y_d = nc.dram_tensor('y', [T // 2, C], f32, kind='ExternalOutput')

    with tile.TileContext(nc) as tc:
        with contextlib.ExitStack() as ctx:
            persist = ctx.enter_context(tc.tile_pool(name='persist', bufs=1))
            dram = ctx.enter_context(tc.tile_pool(name='dram', bufs=1,
                                                  space='DRAM'))

            # ---- constants ----
            identb = persist.tile([128, 128], bf16)
            make_identity(nc, identb)
            ones256 = persist.tile([128, 256], fp8)
            nc.vector.memset(ones256[:], 1.0)
            zero_sb = persist.tile([128, 1], f32)
            nc.vector.memset(zero_sb[:], 0.0)
            ebias_sb = persist.tile([128, 1], f32)
            nc.vector.memset(ebias_sb[:], EXP_BIAS)
            scr_sq = persist.tile([128, C], bf16)   # Square-output scratch
            # residual x2 for my first 4 tiles, staged during qb3 so phase D
            # can start fc1 immediately (overlapping RS#2)
            x2a = persist.tile([128, 4, C], f32)
            x2b4 = persist.tile([128, 4, C], bf16)

            # DRAM scratch for the collectives -- separate tiles per chunk
            # so oproj writes for chunk B never carry a false WAR dependency
            # on RS#1's read of chunk A
            rs_in_a = dram.tile([T // 2, C], bf16)
            rs_in_b = dram.tile([T // 2, C], bf16)
            rs_out_a = dram.tile([T // 4, C], bf16)
            rs_out_b = dram.tile([T // 4, C], bf16)

            with contextlib.ExitStack() as pab:
                ab = pab.enter_context(tc.tile_pool(name='ab', bufs=1))
                qT = ab.tile([128, HPC, T], bf16)
                kT = ab.tile([128, HPC, T], bf16)
                vd_sb = ab.tile([128, NT, HPC, D], fp8)
                at_p = pab.enter_context(tc.tile_pool(name='at_p', bufs=2))

                # ====== phases A+B ======
                with contextlib.ExitStack() as pin:
                    a_w = pin.enter_context(tc.tile_pool(name='a_w', bufs=1))
                    a_x = pin.enter_context(tc.tile_pool(name='a_x', bufs=3))
                    wq_sb = a_w.tile([128, 10, 3 * CPC], fp8)
                    cs_sb = a_w.tile([128, NT, 4, D], bf16)

                    def load_aw():
                        nc.sync.dma_start(
                            out=wq_sb[:],
                            in_=wqkv_d.ap().rearrange('(j p) c -> p j c',
                                                      p=128))
                        nc.sync.dma_start(
                            out=cs_sb[:],
                            in_=cossin_d.ap().rearrange('(t p) x -> p t x',
                                                        p=128))

                    a_t = pin.enter_context(tc.tile_pool(name='a_t', bufs=2))
                    a_s = pin.enter_context(tc.tile_pool(name='a_s', bufs=2))
                    a_q = pin.enter_context(tc.tile_pool(name='a_q', bufs=4))
                    pT_pool = pin.enter_context(
                        tc.tile_pool(name='pT', bufs=4))
                    b_t = pin.enter_context(tc.tile_pool(name='b_t', bufs=2))
                    ps512 = pin.enter_context(
                        tc.tile_pool(name='ps512', bufs=3, space='PSUM'))
                    ops_ps = pin.enter_context(
                        tc.tile_pool(name='ops_ps', bufs=2, space='PSUM'))
                    psT = pin.enter_context(
                        tc.tile_pool(name='psT', bufs=1, space='PSUM'))
                    psD = pin.enter_context(
                        tc.tile_pool(name='psD', bufs=2, space='PSUM'))

                    stash = {}

                    def emit_head(t, first=False):
                        # rstd of x for this token tile (from bf16 x rows)
                        xt = a_s.tile([128, C], bf16, tag='xt')
                        nc.sync.dma_start(
                            out=xt[:], in_=xsq_d[t * 128:(t + 1) * 128, :])
                        xTt = a_x.tile([128, 10, 128], fp8, tag='xTt')
                        nc.sync.dma_start(
                            out=xTt[:],
                            in_=xT_d[:, t * 128:(t + 1) * 128]
                            .rearrange('(j p) t -> p j t', p=128))
                        if first:
                            load_aw()
                        ssq = a_s.tile([128, 1], f32, tag='ssq')
                        nc.scalar.activation(out=scr_sq[:], in_=xt[:],
                                             func=AF.Square, bias=zero_sb[:],
                                             accum_out=ssq[:])
                        # rstd scaled by 2^-15 to undo x*32 and w*1024 fp8
                        # scaling: rsqrt((ssq/C + EPS) * 2^30)
                        rstd = a_s.tile([128, 1], f32, tag='rstd')
                        _rsqrt_vec(nc, a_s, rstd[:], ssq[:],
                                   float((2.0 ** 30) / C),
                                   float(EPS * (2.0 ** 30)), 'rx')
                        # QKV (chunk-outer, j-mid, g-inner: LDW amortized)
                        qf = a_q.tile([128, CPC], bf16, tag='qf')
                        kf = a_q.tile([128, CPC], bf16, tag='kf')
                        vf = a_q.tile([128, CPC], bf16, tag='vf')
                        dsts = (qf, kf, vf)
                        for lo, hi in ((0, 512), (512, 640)):
                            pss = [ps512.tile([128, 512], f32, tag='mm',
                                              name='qkvps')
                                   for _ in range(3)]
                            for jp in range(5):
                                for g in range(3):
                                    nc.tensor.matmul(
                                        pss[g][:, 0:hi - lo],
                                        xTt[:, 2 * jp:2 * jp + 2, :],
                                        wq_sb[:, 2 * jp:2 * jp + 2,
                                              g * CPC + lo:g * CPC + hi],
                                        start=(jp == 0), stop=(jp == 4),
                                        perf_mode=DR)
                            for g in range(3):
                                if g == 0:
                                    nc.vector.tensor_scalar_mul(
                                        out=dsts[g][:, lo:hi],
                                        in0=pss[g][:, 0:hi - lo],
                                        scalar1=rstd[:])
                                else:
                                    nc.scalar.activation(
                                        out=dsts[g][:, lo:hi],
                                        in_=pss[g][:, 0:hi - lo],
                                        func=AF.Copy, scale=rstd[:])
                        stash[t] = (qf, kf, vf)

                    def rope(eng, src, cos_t, sin_t, out):
                        # out[p,h,d] = src*cos + swap(src)*sinneg   (bf16)
                        src3 = src[:].rearrange('p (h d) -> p h d', h=HPC)
                        pa = list(src3.ap)
                        swap = _ap(src3, 64, pa[:2] + [[-64, 2], [1, 64]])
                        ca = list(cos_t.ap)
                        cos4 = _ap(cos_t, 0, [ca[0], [0, HPC], [1, 128]])
                        sin4 = _ap(sin_t, 0,
                                   [ca[0], [0, HPC], [64, 2], [1, 64]])
                        tmp = a_t.tile([128, HPC, D], bf16, tag='rtmp')
                        eng.tensor_tensor(
                            out=tmp[:].rearrange('p h (u d) -> p h u d', u=2),
                            in0=swap, in1=sin4, op=OP.mult)
                        eng.tensor_tensor(out=out[:], in0=src3, in1=cos4,
                                          op=OP.mult)
                        eng.tensor_add(out=out[:], in0=out[:], in1=tmp[:])

                    def hb(ap5, reps):
                        # (128,5) -> (128,5,reps) broadcast
                        a = list(ap5.ap)
                        return bass.AP(tensor=ap5.tensor, offset=ap5.offset,
                                       ap=[a[0], [a[-1][0], HPC], [0, reps]])

                    def emit_tail(t):
                        qf, kf, vf = stash.pop(t)
                        # rms of pre-rope q/k (rope is norm-preserving);
                        # msq64 = 64 * rstd (the fp8 global scale folded in)
                        msq = a_t.tile([128, 2, HPC], f32, tag='msq')
                        qsq = a_t.tile([128, HPC, D], bf16, tag='qsq')
                        qf3 = qf[:].rearrange('p (h d) -> p h d', h=HPC)
                        nc.vector.tensor_tensor(out=qsq[:], in0=qf3,
                                                in1=qf3, op=OP.mult)
                        nc.vector.tensor_reduce(out=msq[:, 0, :],
                                                in_=qsq[:], axis=AX.X,
                                                op=OP.add)
                        ksq = a_t.tile([128, HPC, D], bf16, tag='ksq')
                        kf3 = kf[:].rearrange('p (h d) -> p h d', h=HPC)
                        nc.vector.tensor_tensor(out=ksq[:], in0=kf3,
                                                in1=kf3, op=OP.mult)
                        nc.vector.tensor_reduce(out=msq[:, 1, :],
                                                in_=ksq[:], axis=AX.X,
                                                op=OP.add)
                        _rsqrt_vec(nc, a_t, msq[:], msq[:],
                                   float(1.0 / (D * QSCALE * QSCALE)),
                                   float(EPS / (QSCALE * QSCALE)), 'rqk')
                        # rope (q on vector, k on gpsimd)
                        zq = a_t.tile([128, HPC, D], bf16, tag='zq')
                        rope(nc.vector, qf, cs_sb[:, t, 0, :],
                             cs_sb[:, t, 1, :], zq)
                        zk = a_t.tile([128, HPC, D], bf16, tag='zk')
                        rope(nc.vector, kf, cs_sb[:, t, 2, :],
                             cs_sb[:, t, 3, :], zk)
                        # quantize: one op per tensor (global-scale e4m3)
                        q8v = a_t.tile([128, HPC, D], fp8, tag='q8')
                        nc.vector.tensor_tensor(out=q8v[:], in0=zq[:],
                                                in1=hb(msq[:, 0, :], D),
                                                op=OP.mult)
                        k8v = a_t.tile([128, HPC, D], fp8, tag='k8')
                        nc.vector.tensor_tensor(out=k8v[:], in0=zk[:],
                                                in1=hb(msq[:, 1, :], D),
                                                op=OP.mult)
                        nc.vector.tensor_scalar_mul(
                            out=vd_sb[:, t, :, :],
                            in0=vf[:].rearrange('p (h d) -> p h d', h=HPC),
                            scalar1=QSCALE)
                        # cast quantized q/k back to bf16 (exact) for the
                        # PE transposes (walrus rejects fp8 transposes)
                        qdb = a_t.tile([128, HPC, D], bf16, tag='qdb')
                        nc.scalar.copy(out=qdb[:], in_=q8v[:])
                        kdb = a_t.tile([128, HPC, D], bf16, tag='kdb')
                        nc.scalar.copy(out=kdb[:], in_=k8v[:])
                        for src, dstT in ((qdb, qT), (kdb, kT)):
                            tp = psT.tile([128, 640], bf16, tag='tp')
                            for h in range(HPC):
                                nc.tensor.transpose(
                                    tp[:, h * 128:(h + 1) * 128],
                                    src[:, h, :], identb[:])
                            nc.vector.tensor_copy(
                                out=dstT[:, :, t * 128:(t + 1) * 128],
                                in_=tp[:].rearrange('p (h d) -> p h d',
                                                    h=HPC))

                    def emit_attn_h(qb, h, at):
                        nkt = 4 * qb + 4
                        dps = psD.tile([128, 512], f32, tag='dps')
                        ops = ops_ps.tile([128, 512], f32, tag='ops')
                        for kp in range(nkt // 2):
                            pT2 = pT_pool.tile([128, 2, 512], fp8, tag='pT')
                            for u in range(2):
                                kt = 2 * kp + u
                                sp = ps512.tile([128, 512], f32, tag='mm')
                                nc.tensor.matmul(
                                    sp[:],
                                    kT[:, h, kt * 128:(kt + 1) * 128],
                                    qT[:, h, qb * 512:(qb + 1) * 512],
                                    start=True, stop=True)
                                nc.scalar.activation(
                                    out=pT2[:, u, :], in_=sp[:],
                                    func=AF.Exp, bias=ebias_sb[:],
                                    scale=float(INV_SQRT_D /
                                                (QSCALE * QSCALE)))
                                o = kt - 4 * qb
                                if o >= 0:
                                    nc.gpsimd.affine_select(
                                        out=pT2[:, u, :], in_=pT2[:, u, :],
                                        compare_op=OP.is_ge, fill=0.0,
                                        base=-128 * o, pattern=[[1, 512]],
                                        channel_multiplier=-1)
                            st = (kp == 0)
                            sp_ = (kp == nkt // 2 - 1)
                            nc.tensor.matmul(
                                dps[:],
                                ones256[:].rearrange('p (u m) -> p u m', u=2),
                                pT2[:], start=st, stop=sp_, perf_mode=DR)
                            nc.tensor.matmul(
                                ops[:], vd_sb[:, 2 * kp:2 * kp + 2, h, :],
                                pT2[:], start=st, stop=sp_, perf_mode=DR)
                        rd = b_t.tile([128, 512], f32, tag='rd')
                        nc.vector.reciprocal_approx_fast(out=rd[:],
                                                         in_=dps[:])
                        nc.vector.tensor_tensor(
                            out=at[:, h, :],
                            in0=ops[:], in1=rd[:], op=OP.mult)

                    wo_sb = a_w.tile([128, HPC, C], fp8)
                    nc.sync.dma_start(
                        out=wo_sb[:],
                        in_=wout_d.ap().rearrange('(h p) c -> p h c', p=128))
                    grp = [[2 * i, 2 * i + 1] for i in range(n_cores // 2)]

                    OSC = float(2.0 ** -15)   # undo attn*32 and wo*1024

                    def oproj(tt, at):
                        tl = (tt % 4) * 128
                        ob = b_t.tile([128, C], bf16, tag='ob', name='ob')
                        for ci, (lo, hi) in enumerate(((0, 512),
                                                       (512, 1024),
                                                       (1024, C))):
                            ps = ps512.tile([128, 512], f32, tag='mm',
                                            name='oprojps')
                            for hp2 in range(2):
                                nc.tensor.matmul(
                                    ps[:, 0:hi - lo],
                                    at[:, 2 * hp2:2 * hp2 + 2, tl:tl + 128],
                                    wo_sb[:, 2 * hp2:2 * hp2 + 2, lo:hi],
                                    start=(hp2 == 0), stop=False,
                                    perf_mode=DR)
                            nc.tensor.matmul(
                                ps[:, 0:hi - lo], at[:, 4, tl:tl + 128],
                                wo_sb[:, 4, lo:hi],
                                start=False, stop=True)
                            if ci == 2:
                                nc.scalar.activation(
                                    out=ob[:, lo:hi],
                                    in_=ps[:, 0:hi - lo],
                                    func=AF.Copy, scale=OSC)
                            else:
                                nc.vector.tensor_scalar_mul(
                                    out=ob[:, lo:hi],
                                    in0=ps[:, 0:hi - lo], scalar1=OSC)
                        rs_dst = rs_in_a if tt < 8 else rs_in_b
                        r = tt if tt < 8 else tt - 8
                        nc.sync.dma_start(
                            out=rs_dst[r * 128:(r + 1) * 128, :], in_=ob[:])

                    # ---- interleaved A+B+C emission ----
                    # attn/oproj units of query-block qb are dripped across
                    # the next 4 head/tail slots so the PE FIFO always has
                    # QKV work queued behind exp-gated score tiles.
                    # RS#1 (tiles 0-7) fires after qb=1's oproj; RS#2 at end.
                    pending = []

                    def drip(n):
                        for _ in range(n):
                            if pending:
                                pending.pop(0)()

                    def mk_attn(qb, h, at):
                        return lambda: emit_attn_h(qb, h, at)

                    def mk_oproj(tt, at):
                        return lambda: oproj(tt, at)

                    def mk_rs1():
                        def f():
                            nc.gpsimd.collective_compute(
                                'ReduceScatter', OP.add,
                                ins=[rs_in_a[0:1024, :].opt()],
                                outs=[rs_out_a[0:512, :].opt()],
                                replica_groups=grp)
                        return f

                    for t in range(NT):
                        emit_head(t, first=(t == 0))
                        if t >= 1:
                            emit_tail(t - 1)
                        drip(3)
                        if t >= 4 and t % 4 == 0:
                            qb = t // 4 - 1
                            at = at_p.tile([128, HPC, 512], fp8, tag='at')
                            for h in range(HPC):
                                pending.append(mk_attn(qb, h, at))
                            if qb == 2:
                                # RS#1 trigger goes on the gpsimd queue; it
                                # must come after qb2's affine_selects there
                                # or they stall behind the CC completion.
                                pending.append(mk_rs1())
                            for tt in range(4 * qb, 4 * qb + 4):
                                pending.append(mk_oproj(tt, at))
                    def mlp_pre(tt):
                        # x2 = rs_out + xh for my tiles 0-3 (RS#1 data)
                        rsx = b_t.tile([128, C], bf16, tag='rsx')
                        nc.sync.dma_start(
                            out=rsx[:],
                            in_=rs_out_a[tt * 128:(tt + 1) * 128, :])
                        xht = b_t.tile([128, C], f32, tag='xht')
                        nc.sync.dma_start(
                            out=xht[:],
                            in_=xh_d[tt * 128:(tt + 1) * 128, :])
                        nc.vector.tensor_add(out=x2a[:, tt, :],
                                             in0=rsx[:], in1=xht[:])
                        nc.vector.tensor_copy(out=x2b4[:, tt, :],
                                              in_=x2a[:, tt, :])

                    emit_tail(NT - 1)
                    drip(len(pending))
                    at = at_p.tile([128, HPC, 512], fp8, tag='at')
                    for h in range(HPC):
                        emit_attn_h(QB - 1, h, at)
                    for tt in range(12, 16):
                        oproj(tt, at)
                    nc.gpsimd.collective_compute(
                        'ReduceScatter', OP.add,
                        ins=[rs_in_b[0:1024, :].opt()],
                        outs=[rs_out_b[0:512, :].opt()],
                        replica_groups=grp)
                    # staged residual adds LAST so their RS#1-gated DMAs
                    # never head-of-line-block the vector queue
                    for tt in range(4):
                        mlp_pre(tt)

            # ====== phase D: residual + MLP over my T/2 tokens ======
            # half 0 (tiles 0-3) depends only on RS#1 -> overlaps RS#2.
            with contextlib.ExitStack() as pd:
                d_t = pd.enter_context(tc.tile_pool(name='d_t', bufs=2))
                d_big = pd.enter_context(tc.tile_pool(name='d_big', bufs=1))
                x2_sb = d_big.tile([128, 4, C], f32)
                xn2T = d_big.tile([128, 10, T // 2], bf16)
                h2T = d_big.tile([128, F // 128, T // 2], bf16)
                rinv_sb = d_big.tile([128, NH], f32)

                with tc.tile_pool(name='d_ps', bufs=4, space='PSUM') as d_ps, \
                     tc.tile_pool(name='dt_ps', bufs=2,
                                  space='PSUM') as dt_ps, \
                     tc.tile_pool(name='wf1', bufs=5) as wf1_pool:

                    def x2ap(tt):
                        return x2a[:, tt, :] if tt < 4 \
                            else x2_sb[:, tt - 4, :]

                    def mlp_head(tt):
                        # x2 = rs_out + xh; rstd; transposes into xn2T
                        if tt < 4:
                            x2b = x2b4[:, tt, :]
                        else:
                            rsx = d_t.tile([128, C], bf16, tag='rsx')
                            nc.sync.dma_start(
                                out=rsx[:],
                                in_=rs_out_b[(tt - 4) * 128:(tt - 3) * 128,
                                             :])
                            xht = d_t.tile([128, C], f32, tag='xht')
                            nc.sync.dma_start(
                                out=xht[:],
                                in_=xh_d[tt * 128:(tt + 1) * 128, :])
                            nc.vector.tensor_add(out=x2_sb[:, tt - 4, :],
                                                 in0=rsx[:], in1=xht[:])
                            x2bt = d_t.tile([128, C], bf16, tag='x2b')
                            nc.vector.tensor_copy(out=x2bt[:],
                                                  in_=x2_sb[:, tt - 4, :])
                            x2b = x2bt[:]
                        ssq2 = d_t.tile([128, 1], f32, tag='ssq2')
                        nc.scalar.activation(out=scr_sq[:],
                                             in_=x2ap(tt),
                                             func=AF.Square, bias=zero_sb[:],
                                             accum_out=ssq2[:])
                        m2 = d_t.tile([128, 1], f32, tag='m2')
                        nc.vector.tensor_scalar(out=m2[:], in0=ssq2[:],
                                                scalar1=float(1.0 / C),
                                                scalar2=EPS,
                                                op0=OP.mult, op1=OP.add)
                        nc.vector.reciprocal_approx_fast(
                            out=rinv_sb[:, tt:tt + 1], in_=m2[:])
                        for jg, (lo, hi) in enumerate(((0, 4), (4, 8),
                                                      (8, 10))):
                            tp2 = dt_ps.tile([128, 512], bf16, tag='tp2')
                            for j in range(lo, hi):
                                nc.tensor.transpose(
                                    tp2[:, (j - lo) * 128:(j - lo + 1) * 128],
                                    x2b[:, j * 128:(j + 1) * 128],
                                    identb[:])
                            nc.vector.tensor_copy(
                                out=xn2T[:, lo:hi, tt * 128:(tt + 1) * 128],
                                in_=tp2[:, 0:(hi - lo) * 128].rearrange(
                                    'p (j d) -> p j d', d=128))

                    wf1_pre = {}

                    def wf1_fetch(fi):
                        wf1 = wf1_pool.tile([128, 10, 128], bf16,
                                            tag='wf1')
                        nc.sync.dma_start(
                            out=wf1[:],
                            in_=wfc1_d[:, fi * 128:(fi + 1) * 128]
                            .rearrange('(j p) c -> p j c', p=128))
                        return wf1

                    def fc1_half(half):
                        lo_t = half * 512
                        for fi in range(F // 128):
                            wf1 = wf1_pre.pop(fi, None) if half == 0 \
                                else None
                            if wf1 is None:
                                wf1 = wf1_fetch(fi)
                            hp = d_ps.tile([128, 512], f32, tag='hps')
                            for j in range(10):
                                nc.tensor.matmul(
                                    hp[:], wf1[:, j, :],
                                    xn2T[:, j, lo_t:lo_t + 512],
                                    start=(j == 0), stop=(j == 9))
                            hrelu = d_t.tile([128, 512], bf16, tag='hrelu')
                            nc.scalar.activation(out=hrelu[:], in_=hp[:],
                                                 func=AF.Relu,
                                                 bias=zero_sb[:])
                            nc.vector.tensor_mul(
                                out=h2T[:, fi, lo_t:lo_t + 512],
                                in0=hrelu[:], in1=hrelu[:])

                    for fi in range(4):
                        wf1_pre[fi] = wf1_fetch(fi)
                    for tt in range(4):
                        mlp_head(tt)
                    fc1_half(0)
                    for tt in range(4, 8):
                        mlp_head(tt)
                    fc1_half(1)

                # fc2: c-halves x tt-quads; 2 matmuls (640 cols) per lhsT
                with tc.tile_pool(name='y_ps', bufs=4, space='PSUM') as y_ps, \
                     tc.tile_pool(name='wf2', bufs=3) as wf2_pool:
                    for clo, chi in ((0, 640), (640, C)):
                        for ttg in range(2):
                            yps = [y_ps.tile([128, 640], f32, tag='yps',
                                             name='yps')
                                   for _ in range(4)]
                            for f2 in range(F // 256):
                                wf2 = wf2_pool.tile([128, 2, 640], bf16,
                                                    tag='wf2')
                                nc.sync.dma_start(
                                    out=wf2[:],
                                    in_=wfc2_d[f2 * 256:(f2 + 1) * 256,
                                               clo:chi]
                                    .rearrange('(u p) c -> p u c', p=128))
                                for u in range(2):
                                    fi = 2 * f2 + u
                                    st = (fi == 0)
                                    sp_ = (fi == F // 128 - 1)
                                    for i in range(4):
                                        tt = 4 * ttg + i
                                        lhsT = h2T[:, fi,
                                                   tt * 128:(tt + 1) * 128]
                                        nc.tensor.matmul(
                                            yps[i][:, 0:512], lhsT,
                                            wf2[:, u, 0:512],
                                            start=st, stop=sp_)
                                        nc.tensor.matmul(
                                            yps[i][:, 512:640], lhsT,
                                            wf2[:, u, 512:640],
                                            start=st, stop=sp_)
                            for i in range(4):
                                tt = 4 * ttg + i
                                yo = d_t.tile([128, 640], f32, tag='yo')
                                nc.vector.scalar_tensor_tensor(
                                    out=yo[:], in0=yps[i][:],
                                    scalar=rinv_sb[:, tt:tt + 1],
                                    in1=x2ap(tt)[:, clo:chi],
                                    op0=OP.mult, op1=OP.add)
                                nc.sync.dma_start(
                                    out=y_d[tt * 128:(tt + 1) * 128,
                                            clo:chi],
                                    in_=yo[:])

    nc.compile()
    return nc


_CACHE = {}


def _get_nc(t_len=T):
    if t_len not in _CACHE:
        _CACHE[t_len] = build_nc(t_len)
    return _CACHE[t_len]


def make_in_maps(x, rotary_pos_emb, ln1_w, w_qkv, qn_w, kn_w, w_out, ln2_w,
                 w_fc1, w_fc2, t_len=T):
    """Host-side sharding prep. Returns list of per-core input dicts."""
    x = np.asarray(x, np.float32)
    rot = np.asarray(rotary_pos_emb, np.float32)
    cos = np.cos(rot).astype(np.float32)
    sin = np.sin(rot).astype(np.float32)
    sinneg = np.concatenate([-sin[:, :64], sin[:, :64]], axis=-1)
    qn = np.asarray(qn_w, np.float32)
    kn = np.asarray(kn_w, np.float32)
    cossin = np.stack([cos * qn, sinneg * qn, cos * kn, sinneg * kn],
                      axis=1).reshape(T, 4 * D).astype(ml_dtypes.bfloat16)
    cossin = np.ascontiguousarray(cossin)
    w_qkv_f = (np.asarray(w_qkv, np.float32) * 1024.0
               * np.asarray(ln1_w, np.float32)[:, None]).reshape(C, 3, H, D)
    w_fc1_f = (np.asarray(w_fc1, np.float32)
               * np.asarray(ln2_w, np.float32)[:, None]
               ).astype(ml_dtypes.bfloat16)
    w_fc2_b = np.asarray(w_fc2, np.float32).astype(ml_dtypes.bfloat16)
    # attn carries a *32 scale; w_out carries *1024 fp8 scale (undone by
    # the 2^-15 psum copy scale on-core)
    wo = (np.asarray(w_out, np.float32) * 32768.0 / QSCALE).reshape(H, D, C)

    # tokens owned per rank within a pair: rank0 tiles {0-3, 8-11},
    # rank1 tiles {4-7, 12-15} (tile = 128 tokens)
    halves = [np.r_[0:512, 1024:1536], np.r_[512:1024, 1536:2048]]

    in_maps = []
    for c in range(N_CORES):
        b, hg = c // 2, c % 2
        heads = slice(hg * HPC, (hg + 1) * HPC)
        wq = np.ascontiguousarray(
            w_qkv_f[:, :, heads, :].reshape(C, 3 * CPC)
        ).astype(ml_dtypes.float8_e4m3)
        w_outp = np.ascontiguousarray(
            wo[heads].reshape(CPC, C)).astype(ml_dtypes.float8_e4m3)
        xb = x[b]
        in_maps.append({
            'xsq': np.ascontiguousarray(xb).astype(ml_dtypes.bfloat16),
            'xT': np.ascontiguousarray(
                xb.T * QSCALE).astype(ml_dtypes.float8_e4m3),
            'xh': np.ascontiguousarray(xb[halves[hg]]),
            'w_qkv': wq,
            'cossin': cossin,
            'w_out': w_outp,
            'w_fc1': np.ascontiguousarray(w_fc1_f),
            'w_fc2': np.ascontiguousarray(w_fc2_b),
        })
    return in_maps


def assemble_output(results, t_len=T):
    halves = [np.r_[0:512, 1024:1536], np.r_[512:1024, 1536:2048]]
    out = np.zeros((B, t_len, C), np.float32)
    for c in range(N_CORES):
        b, hg = c // 2, c % 2
        out[b, halves[hg]] = results[c]['y']
    return out


def kernel(**inputs):
    nc = _get_nc(T)
    in_maps = make_in_maps(**inputs)
    res = bass_utils.run_bass_kernel_spmd(nc, in_maps,
                                          core_ids=list(range(N_CORES)))
    return assemble_output(res.results)


# revision 36
# speedup vs baseline: 1.0383x; 1.0214x over previous
"""Trainium2 Bass kernel for nn_Block_27848567948000 (dense transformer block).

Sharding (8 NeuronCores): 4 data-parallel groups over batch (B=4), 2-way
tensor-parallel within each pair: attention sharded over heads (5 each).
out_proj computed as per-head partial sums over ALL T, summed + token-scattered
via pairwise ReduceScatters; MLP over the core's T/2 tokens.

Token ownership (per pair): rank0 owns tiles {0-3, 8-11}, rank1 owns
{4-7, 12-15}, so ReduceScatter #1 (rows 0:1024 = tiles 0-7) can fire right
after query-block qb=1 and RS#2 after qb=3; fc1 on the first half overlaps
RS#2.

Quantization: the reference's mxfp8 QDQ equals a plain e4m3 cast under a
global power-of-2 scale for all values in fp8-normal range (validated
numerically: rel err ~2e-3 incl. fp8 softmax probs).  q/k are quantized with
scale 32*rstd (rms fold), v with 32; w_out pre-scaled by 1/32 on the host.
NOTE dt.float8e4 is IEEE e4m3: max finite 240, inf above -- scale 32 keeps
|q*32|<=170, |v*32|<=196, and exp bias -1 keeps p<=80.  Softmax probs are fp8,
enabling DoubleRow (2x fp8) matmuls for P@V + denominator over kt-tile pairs.

kernel(**inputs) takes FULL inputs and returns the FULL (4, 2048, 1280) output.
"""
import sys

sys.path.insert(0, '/opt/trn_rl_repo')

import numpy as np
import ml_dtypes

import concourse.bass as bass
import concourse.tile as tile
from concourse import mybir, bacc
from concourse import bass_utils
from concourse.masks import make_identity

B, T, C, H, D, F = 4, 2048, 1280, 10, 128, 5120
EPS = 1e-5
N_CORES = 8
HPC = H // 2            # heads per core (5)
CPC = HPC * D           # channels per core (640)
f32 = mybir.dt.float32
bf16 = mybir.dt.bfloat16
fp8 = mybir.dt.float8e4
i32 = mybir.dt.int32
AF = mybir.ActivationFunctionType
OP = mybir.AluOpType
AX = mybir.AxisListType
DR = mybir.MatmulPerfMode.DoubleRow

NT = T // 128            # 16 token tiles
NH = T // 2 // 128       # 8 token tiles in my half
QB = T // 512            # 4 query blocks
INV_SQRT_D = float(1.0 / np.sqrt(D))
EXP_BIAS = -1.0
QSCALE = 32.0   # global fp8 scale for q/k/v (e4m3 max finite is 240!)


def _rsqrt_vec(nc, pool, out_ap, in_ap, scale, eps, tag, eng=None):
    """out = 1/sqrt(in*scale + eps) on a DVE-like engine (no act tables).
    Bit-trick seed + 2 Newton iterations (~1e-6 rel err). Shapes (128, n)."""
    if eng is None:
        eng = nc.vector
    i32_ = mybir.dt.int32
    shp = [128, in_ap.free_size()]
    m = pool.tile(shp, f32, tag=tag + 'm', name='rs_m')
    eng.tensor_scalar(out=m[:], in0=in_ap, scalar1=scale, scalar2=eps,
                      op0=OP.mult, op1=OP.add)
    y = pool.tile(shp, f32, tag=tag + 'y', name='rs_y')
    eng.tensor_single_scalar(out=y[:].bitcast(i32_),
                             in_=m[:].bitcast(i32_), scalar=1,
                             op=OP.logical_shift_right)
    eng.tensor_scalar(out=y[:].bitcast(i32_), in0=y[:].bitcast(i32_),
                      scalar1=-1, scalar2=0x5f3759df,
                      op0=OP.mult, op1=OP.add)
    t = pool.tile(shp, f32, tag=tag + 't', name='rs_t')
    for it in range(2):
        eng.tensor_tensor(out=t[:], in0=y[:], in1=y[:], op=OP.mult)
        eng.tensor_tensor(out=t[:], in0=t[:], in1=m[:], op=OP.mult)
        eng.tensor_scalar(out=t[:], in0=t[:], scalar1=-0.5,
                          scalar2=1.5, op0=OP.mult, op1=OP.add)
        eng.tensor_tensor(out=y[:] if it == 0 else out_ap, in0=y[:],
                          in1=t[:], op=OP.mult)


def _ap(t_ap, offset_delta, pattern):
    return bass.AP(tensor=t_ap.tensor, offset=t_ap.offset + offset_delta,
                   ap=pattern)


def build_nc(t_len=T, n_cores=N_CORES):
    import contextlib
    nc = bacc.Bacc('TRN2', target_bir_lowering=False, debug=False,
                   num_devices=n_cores)

    # ---- DRAM I/O ----
    # xsq: row-major bf16 x (for rmsnorm sum-of-squares)
    # xT:  transposed bf16 x [C, T] (QKV lhsT; no PE transposes needed)
    xsq_d = nc.dram_tensor('xsq', [T, C], bf16, kind='ExternalInput')
    xT_d = nc.dram_tensor('xT', [C, T], fp8, kind='ExternalInput')
    xh_d = nc.dram_tensor('xh', [T // 2, C], f32, kind='ExternalInput')
    wqkv_d = nc.dram_tensor('w_qkv', [C, 3 * CPC], fp8, kind='ExternalInput')
    cossin_d = nc.dram_tensor('cossin', [T, 4 * D], bf16,
                              kind='ExternalInput')
    wout_d = nc.dram_tensor('w_out', [CPC, C], fp8, kind='ExternalInput')
    wfc1_d = nc.dram_tensor('w_fc1', [C, F], bf16, kind='ExternalInput')
    wfc2_d = nc.dram_tensor('w_fc2', [F, C], bf16, kind='ExternalInput')
    y_d = nc.dram_tensor('y', [T // 2, C], f32, kind='ExternalOutput')

    with tile.TileContext(nc) as tc:
        with contextlib.ExitStack() as ctx:
            persist = ctx.enter_context(tc.tile_pool(name='persist', bufs=1))
            dram = ctx.enter_context(tc.tile_pool(name='dram', bufs=1,
                                                  space='DRAM'))

            # ---- constants ----
            identb = persist.tile([128, 128], bf16)
            make_identity(nc, identb)
            ones256 = persist.tile([128, 256], fp8)
            nc.vector.memset(ones256[:], 1.0)
            zero_sb = persist.tile([128, 1], f32)
            nc.vector.memset(zero_sb[:], 0.0)
            ebias_sb = persist.tile([128, 1], f32)
            nc.vector.memset(ebias_sb[:], EXP_BIAS)
            scr_sq = persist.tile([128, C], bf16)   # Square-output scratch

            # DRAM scratch for the collectives -- separate tiles per chunk
            # so oproj writes for chunk B never carry a false WAR dependency
            # on RS#1's read of chunk A
            rs_in_a = dram.tile([T // 2, C], bf16)
            rs_in_b = dram.tile([T // 2, C], bf16)
            rs_out_a = dram.tile([T // 4, C], bf16)
            rs_out_b = dram.tile([T // 4, C], bf16)

            with contextlib.ExitStack() as pab:
                ab = pab.enter_context(tc.tile_pool(name='ab', bufs=1))
                qT = ab.tile([128, HPC, T], bf16)
                kT = ab.tile([128, HPC, T], bf16)
                vd_sb = ab.tile([128, NT, HPC, D], fp8)
                at_p = pab.enter_context(tc.tile_pool(name='at_p', bufs=2))

                # ====== phases A+B ======
                with contextlib.ExitStack() as pin:
                    a_w = pin.enter_context(tc.tile_pool(name='a_w', bufs=1))
                    a_x = pin.enter_context(tc.tile_pool(name='a_x', bufs=3))
                    wq_sb = a_w.tile([128, 10, 3 * CPC], fp8)
                    cs_sb = a_w.tile([128, NT, 4, D], bf16)

                    def load_aw():
                        nc.sync.dma_start(
                            out=wq_sb[:],
                            in_=wqkv_d.ap().rearrange('(j p) c -> p j c',
                                                      p=128))
                        nc.sync.dma_start(
                            out=cs_sb[:],
                            in_=cossin_d.ap().rearrange('(t p) x -> p t x',
                                                        p=128))

                    a_t = pin.enter_context(tc.tile_pool(name='a_t', bufs=2))
                    a_s = pin.enter_context(tc.tile_pool(name='a_s', bufs=2))
                    a_q = pin.enter_context(tc.tile_pool(name='a_q', bufs=4))
                    pT_pool = pin.enter_context(
                        tc.tile_pool(name='pT', bufs=4))
                    b_t = pin.enter_context(tc.tile_pool(name='b_t', bufs=2))
                    ps512 = pin.enter_context(
                        tc.tile_pool(name='ps512', bufs=3, space='PSUM'))
                    ops_ps = pin.enter_context(
                        tc.tile_pool(name='ops_ps', bufs=2, space='PSUM'))
                    psT = pin.enter_context(
                        tc.tile_pool(name='psT', bufs=1, space='PSUM'))
                    psD = pin.enter_context(
                        tc.tile_pool(name='psD', bufs=2, space='PSUM'))

                    stash = {}

                    def emit_head(t, first=False):
                        # rstd of x for this token tile (from bf16 x rows)
                        xt = a_s.tile([128, C], bf16, tag='xt')
                        nc.sync.dma_start(
                            out=xt[:], in_=xsq_d[t * 128:(t + 1) * 128, :])
                        xTt = a_x.tile([128, 10, 128], fp8, tag='xTt')
                        nc.sync.dma_start(
                            out=xTt[:],
                            in_=xT_d[:, t * 128:(t + 1) * 128]
                            .rearrange('(j p) t -> p j t', p=128))
                        if first:
                            load_aw()
                        ssq = a_s.tile([128, 1], f32, tag='ssq')
                        nc.scalar.activation(out=scr_sq[:], in_=xt[:],
                                             func=AF.Square, bias=zero_sb[:],
                                             accum_out=ssq[:])
                        # rstd scaled by 2^-15 to undo x*32 and w*1024 fp8
                        # scaling: rsqrt((ssq/C + EPS) * 2^30)
                        rstd = a_s.tile([128, 1], f32, tag='rstd')
                        _rsqrt_vec(nc, a_s, rstd[:], ssq[:],
                                   float((2.0 ** 30) / C),
                                   float(EPS * (2.0 ** 30)), 'rx')
                        # QKV (chunk-outer, j-mid, g-inner: LDW amortized)
                        qf = a_q.tile([128, CPC], bf16, tag='qf')
                        kf = a_q.tile([128, CPC], bf16, tag='kf')
                        vf = a_q.tile([128, CPC], bf16, tag='vf')
                        dsts = (qf, kf, vf)
                        for lo, hi in ((0, 512), (512, 640)):
                            pss = [ps512.tile([128, 512], f32, tag='mm',
                                              name='qkvps')
                                   for _ in range(3)]
                            for jp in range(5):
                                for g in range(3):
                                    nc.tensor.matmul(
                                        pss[g][:, 0:hi - lo],
                                        xTt[:, 2 * jp:2 * jp + 2, :],
                                        wq_sb[:, 2 * jp:2 * jp + 2,
                                              g * CPC + lo:g * CPC + hi],
                                        start=(jp == 0), stop=(jp == 4),
                                        perf_mode=DR)
                            for g in range(3):
                                if g == 0:
                                    nc.vector.tensor_scalar_mul(
                                        out=dsts[g][:, lo:hi],
                                        in0=pss[g][:, 0:hi - lo],
                                        scalar1=rstd[:])
                                else:
                                    nc.scalar.activation(
                                        out=dsts[g][:, lo:hi],
                                        in_=pss[g][:, 0:hi - lo],
                                        func=AF.Copy, scale=rstd[:])
                        stash[t] = (qf, kf, vf)

                    def rope(eng, src, cos_t, sin_t, out):
                        # out[p,h,d] = src*cos + swap(src)*sinneg   (bf16)
                        src3 = src[:].rearrange('p (h d) -> p h d', h=HPC)
                        pa = list(src3.ap)
                        swap = _ap(src3, 64, pa[:2] + [[-64, 2], [1, 64]])
                        ca = list(cos_t.ap)
                        cos4 = _ap(cos_t, 0, [ca[0], [0, HPC], [1, 128]])
                        sin4 = _ap(sin_t, 0,
                                   [ca[0], [0, HPC], [64, 2], [1, 64]])
                        tmp = a_t.tile([128, HPC, D], bf16, tag='rtmp')
                        eng.tensor_tensor(
                            out=tmp[:].rearrange('p h (u d) -> p h u d', u=2),
                            in0=swap, in1=sin4, op=OP.mult)
                        eng.tensor_tensor(out=out[:], in0=src3, in1=cos4,
                                          op=OP.mult)
                        eng.tensor_add(out=out[:], in0=out[:], in1=tmp[:])

                    def hb(ap5, reps):
                        # (128,5) -> (128,5,reps) broadcast
                        a = list(ap5.ap)
                        return bass.AP(tensor=ap5.tensor, offset=ap5.offset,
                                       ap=[a[0], [a[-1][0], HPC], [0, reps]])

                    def emit_tail(t):
                        qf, kf, vf = stash.pop(t)
                        # rms of pre-rope q/k (rope is norm-preserving);
                        # msq64 = 64 * rstd (the fp8 global scale folded in)
                        msq = a_t.tile([128, 2, HPC], f32, tag='msq')
                        qsq = a_t.tile([128, HPC, D], bf16, tag='qsq')
                        qf3 = qf[:].rearrange('p (h d) -> p h d', h=HPC)
                        nc.vector.tensor_tensor(out=qsq[:], in0=qf3,
                                                in1=qf3, op=OP.mult)
                        nc.vector.tensor_reduce(out=msq[:, 0, :],
                                                in_=qsq[:], axis=AX.X,
                                                op=OP.add)
                        ksq = a_t.tile([128, HPC, D], bf16, tag='ksq')
                        kf3 = kf[:].rearrange('p (h d) -> p h d', h=HPC)
                        nc.vector.tensor_tensor(out=ksq[:], in0=kf3,
                                                in1=kf3, op=OP.mult)
                        nc.vector.tensor_reduce(out=msq[:, 1, :],
                                                in_=ksq[:], axis=AX.X,
                                                op=OP.add)
                        _rsqrt_vec(nc, a_t, msq[:], msq[:],
                                   float(1.0 / (D * QSCALE * QSCALE)),
                                   float(EPS / (QSCALE * QSCALE)), 'rqk')
                        # rope (q on vector, k on gpsimd)
                        zq = a_t.tile([128, HPC, D], bf16, tag='zq')
                        rope(nc.vector, qf, cs_sb[:, t, 0, :],
                             cs_sb[:, t, 1, :], zq)
                        zk = a_t.tile([128, HPC, D], bf16, tag='zk')
                        rope(nc.vector, kf, cs_sb[:, t, 2, :],
                             cs_sb[:, t, 3, :], zk)
                        # quantize: one op per tensor (global-scale e4m3)
                        q8v = a_t.tile([128, HPC, D], fp8, tag='q8')
                        nc.vector.tensor_tensor(out=q8v[:], in0=zq[:],
                                                in1=hb(msq[:, 0, :], D),
                                                op=OP.mult)
                        k8v = a_t.tile([128, HPC, D], fp8, tag='k8')
                        nc.vector.tensor_tensor(out=k8v[:], in0=zk[:],
                                                in1=hb(msq[:, 1, :], D),
                                                op=OP.mult)
                        nc.vector.tensor_scalar_mul(
                            out=vd_sb[:, t, :, :],
                            in0=vf[:].rearrange('p (h d) -> p h d', h=HPC),
                            scalar1=QSCALE)
                        # cast quantized q/k back to bf16 (exact) for the
                        # PE transposes (walrus rejects fp8 transposes)
                        qdb = a_t.tile([128, HPC, D], bf16, tag='qdb')
                        nc.scalar.copy(out=qdb[:], in_=q8v[:])
                        kdb = a_t.tile([128, HPC, D], bf16, tag='kdb')
                        nc.scalar.copy(out=kdb[:], in_=k8v[:])
                        for src, dstT in ((qdb, qT), (kdb, kT)):
                            tp = psT.tile([128, 640], bf16, tag='tp')
                            for h in range(HPC):
                                nc.tensor.transpose(
                                    tp[:, h * 128:(h + 1) * 128],
                                    src[:, h, :], identb[:])
                            nc.vector.tensor_copy(
                                out=dstT[:, :, t * 128:(t + 1) * 128],
                                in_=tp[:].rearrange('p (h d) -> p h d',
                                                    h=HPC))

                    def emit_attn_h(qb, h, at):
                        nkt = 4 * qb + 4
                        dps = psD.tile([128, 512], f32, tag='dps')
                        ops = ops_ps.tile([128, 512], f32, tag='ops')
                        for kp in range(nkt // 2):
                            pT2 = pT_pool.tile([128, 2, 512], fp8, tag='pT')
                            for u in range(2):
                                kt = 2 * kp + u
                                sp = ps512.tile([128, 512], f32, tag='mm')
                                nc.tensor.matmul(
                                    sp[:],
                                    kT[:, h, kt * 128:(kt + 1) * 128],
                                    qT[:, h, qb * 512:(qb + 1) * 512],
                                    start=True, stop=True)
                                nc.scalar.activation(
                                    out=pT2[:, u, :], in_=sp[:],
                                    func=AF.Exp, bias=ebias_sb[:],
                                    scale=float(INV_SQRT_D /
                                                (QSCALE * QSCALE)))
                                o = kt - 4 * qb
                                if o >= 0:
                                    nc.gpsimd.affine_select(
                                        out=pT2[:, u, :], in_=pT2[:, u, :],
                                        compare_op=OP.is_ge, fill=0.0,
                                        base=-128 * o, pattern=[[1, 512]],
                                        channel_multiplier=-1)
                            st = (kp == 0)
                            sp_ = (kp == nkt // 2 - 1)
                            nc.tensor.matmul(
                                dps[:],
                                ones256[:].rearrange('p (u m) -> p u m', u=2),
                                pT2[:], start=st, stop=sp_, perf_mode=DR)
                            nc.tensor.matmul(
                                ops[:], vd_sb[:, 2 * kp:2 * kp + 2, h, :],
                                pT2[:], start=st, stop=sp_, perf_mode=DR)
                        rd = b_t.tile([128, 512], f32, tag='rd')
                        nc.vector.reciprocal_approx_fast(out=rd[:],
                                                         in_=dps[:])
                        nc.vector.tensor_tensor(
                            out=at[:, h, :],
                            in0=ops[:], in1=rd[:], op=OP.mult)

                    wo_sb = a_w.tile([128, HPC, C], fp8)
                    nc.sync.dma_start(
                        out=wo_sb[:],
                        in_=wout_d.ap().rearrange('(h p) c -> p h c', p=128))
                    grp = [[2 * i, 2 * i + 1] for i in range(n_cores // 2)]

                    OSC = float(2.0 ** -15)   # undo attn*32 and wo*1024

                    def oproj(tt, at):
                        tl = (tt % 4) * 128
                        ob = b_t.tile([128, C], bf16, tag='ob', name='ob')
                        for ci, (lo, hi) in enumerate(((0, 512),
                                                       (512, 1024),
                                                       (1024, C))):
                            ps = ps512.tile([128, 512], f32, tag='mm',
                                            name='oprojps')
                            for hp2 in range(2):
                                nc.tensor.matmul(
                                    ps[:, 0:hi - lo],
                                    at[:, 2 * hp2:2 * hp2 + 2, tl:tl + 128],
                                    wo_sb[:, 2 * hp2:2 * hp2 + 2, lo:hi],
                                    start=(hp2 == 0), stop=False,
                                    perf_mode=DR)
                            nc.tensor.matmul(
                                ps[:, 0:hi - lo], at[:, 4, tl:tl + 128],
                                wo_sb[:, 4, lo:hi],
                                start=False, stop=True)
                            if ci == 2:
                                nc.scalar.activation(
                                    out=ob[:, lo:hi],
                                    in_=ps[:, 0:hi - lo],
                                    func=AF.Copy, scale=OSC)
                            else:
                                nc.vector.tensor_scalar_mul(
                                    out=ob[:, lo:hi],
                                    in0=ps[:, 0:hi - lo], scalar1=OSC)
                        rs_dst = rs_in_a if tt < 8 else rs_in_b
                        r = tt if tt < 8 else tt - 8
                        nc.sync.dma_start(
                            out=rs_dst[r * 128:(r + 1) * 128, :], in_=ob[:])

                    # ---- interleaved A+B+C emission ----
                    # attn/oproj units of query-block qb are dripped across
                    # the next 4 head/tail slots so the PE FIFO always has
                    # QKV work queued behind exp-gated score tiles.
                    # RS#1 (tiles 0-7) fires after qb=1's oproj; RS#2 at end.
                    pending = []

                    def drip(n):
                        for _ in range(n):
                            if pending:
                                pending.pop(0)()

                    def mk_attn(qb, h, at):
                        return lambda: emit_attn_h(qb, h, at)

                    def mk_oproj(tt, at):
                        return lambda: oproj(tt, at)

                    def mk_rs1():
                        def f():
                            nc.gpsimd.collective_compute(
                                'ReduceScatter', OP.add,
                                ins=[rs_in_a[0:1024, :].opt()],
                                outs=[rs_out_a[0:512, :].opt()],
                                replica_groups=grp)
                        return f

                    for t in range(NT):
                        emit_head(t, first=(t == 0))
                        if t >= 1:
                            emit_tail(t - 1)
                        drip(3)
                        if t >= 4 and t % 4 == 0:
                            qb = t // 4 - 1
                            at = at_p.tile([128, HPC, 512], fp8, tag='at')
                            for h in range(HPC):
                                pending.append(mk_attn(qb, h, at))
                            if qb == 2:
                                # RS#1 trigger goes on the gpsimd queue; it
                                # must come after qb2's affine_selects there
                                # or they stall behind the CC completion.
                                pending.append(mk_rs1())
                            for tt in range(4 * qb, 4 * qb + 4):
                                pending.append(mk_oproj(tt, at))
                    emit_tail(NT - 1)
                    drip(len(pending))
                    at = at_p.tile([128, HPC, 512], fp8, tag='at')
                    for h in range(HPC):
                        emit_attn_h(QB - 1, h, at)
                    for tt in range(12, 16):
                        oproj(tt, at)
                    nc.gpsimd.collective_compute(
                        'ReduceScatter', OP.add,
                        ins=[rs_in_b[0:1024, :].opt()],
                        outs=[rs_out_b[0:512, :].opt()],
                        replica_groups=grp)

            # ====== phase D: residual + MLP over my T/2 tokens ======
            # half 0 (tiles 0-3) depends only on RS#1 -> overlaps RS#2.
            with contextlib.ExitStack() as pd:
                d_t = pd.enter_context(tc.tile_pool(name='d_t', bufs=2))
                d_big = pd.enter_context(tc.tile_pool(name='d_big', bufs=1))
                x2_sb = d_big.tile([128, NH, C], f32)
                xn2T = d_big.tile([128, 10, T // 2], bf16)
                h2T = d_big.tile([128, F // 128, T // 2], bf16)
                rinv_sb = d_big.tile([128, NH], f32)

                with tc.tile_pool(name='d_ps', bufs=4, space='PSUM') as d_ps, \
                     tc.tile_pool(name='dt_ps', bufs=2,
                                  space='PSUM') as dt_ps, \
                     tc.tile_pool(name='wf1', bufs=5) as wf1_pool:

                    def x2ap(tt):
                        return x2_sb[:, tt, :]

                    def mlp_head(tt):
                        # x2 = rs_out + xh; rstd; transposes into xn2T
                        rsx = d_t.tile([128, C], bf16, tag='rsx')
                        if tt < 4:
                            nc.sync.dma_start(
                                out=rsx[:],
                                in_=rs_out_a[tt * 128:(tt + 1) * 128, :])
                        else:
                            nc.sync.dma_start(
                                out=rsx[:],
                                in_=rs_out_b[(tt - 4) * 128:(tt - 3) * 128,
                                             :])
                        xht = d_t.tile([128, C], f32, tag='xht')
                        nc.sync.dma_start(
                            out=xht[:],
                            in_=xh_d[tt * 128:(tt + 1) * 128, :])
                        nc.vector.tensor_add(out=x2_sb[:, tt, :],
                                             in0=rsx[:], in1=xht[:])
                        x2bt = d_t.tile([128, C], bf16, tag='x2b')
                        nc.vector.tensor_copy(out=x2bt[:],
                                              in_=x2_sb[:, tt, :])
                        x2b = x2bt[:]
                        ssq2 = d_t.tile([128, 1], f32, tag='ssq2')
                        nc.scalar.activation(out=scr_sq[:],
                                             in_=x2ap(tt),
                                             func=AF.Square, bias=zero_sb[:],
                                             accum_out=ssq2[:])
                        m2 = d_t.tile([128, 1], f32, tag='m2')
                        nc.vector.tensor_scalar(out=m2[:], in0=ssq2[:],
                                                scalar1=float(1.0 / C),
                                                scalar2=EPS,
                                                op0=OP.mult, op1=OP.add)
                        nc.vector.reciprocal_approx_fast(
                            out=rinv_sb[:, tt:tt + 1], in_=m2[:])
                        for jg, (lo, hi) in enumerate(((0, 4), (4, 8),
                                                      (8, 10))):
                            tp2 = dt_ps.tile([128, 512], bf16, tag='tp2')
                            for j in range(lo, hi):
                                nc.tensor.transpose(
                                    tp2[:, (j - lo) * 128:(j - lo + 1) * 128],
                                    x2b[:, j * 128:(j + 1) * 128],
                                    identb[:])
                            nc.vector.tensor_copy(
                                out=xn2T[:, lo:hi, tt * 128:(tt + 1) * 128],
                                in_=tp2[:, 0:(hi - lo) * 128].rearrange(
                                    'p (j d) -> p j d', d=128))

                    wf1_pre = {}

                    def wf1_fetch(fi):
                        wf1 = wf1_pool.tile([128, 10, 128], bf16,
                                            tag='wf1')
                        nc.sync.dma_start(
                            out=wf1[:],
                            in_=wfc1_d[:, fi * 128:(fi + 1) * 128]
                            .rearrange('(j p) c -> p j c', p=128))
                        return wf1

                    def fc1_half(half):
                        lo_t = half * 512
                        for fi in range(F // 128):
                            wf1 = wf1_pre.pop(fi, None) if half == 0 \
                                else None
                            if wf1 is None:
                                wf1 = wf1_fetch(fi)
                            hp = d_ps.tile([128, 512], f32, tag='hps')
                            for j in range(10):
                                nc.tensor.matmul(
                                    hp[:], wf1[:, j, :],
                                    xn2T[:, j, lo_t:lo_t + 512],
                                    start=(j == 0), stop=(j == 9))
                            hrelu = d_t.tile([128, 512], bf16, tag='hrelu')
                            nc.scalar.activation(out=hrelu[:], in_=hp[:],
                                                 func=AF.Relu,
                                                 bias=zero_sb[:])
                            nc.vector.tensor_mul(
                                out=h2T[:, fi, lo_t:lo_t + 512],
                                in0=hrelu[:], in1=hrelu[:])

                    for fi in range(4):
                        wf1_pre[fi] = wf1_fetch(fi)
                    for tt in range(4):
                        mlp_head(tt)
                    fc1_half(0)
                    for tt in range(4, 8):
                        mlp_head(tt)
                    fc1_half(1)

                # fc2: c-halves x tt-quads; 2 matmuls (640 cols) per lhsT
                with tc.tile_pool(name='y_ps', bufs=4, space='PSUM') as y_ps, \
                     tc.tile_pool(name='wf2', bufs=3) as wf2_pool:
                    for clo, chi in ((0, 640), (640, C)):
                        for ttg in range(2):
                            yps = [y_ps.tile([128, 640], f32, tag='yps',
                                             name='yps')
                                   for _ in range(4)]
                            for f2 in range(F // 256):
                                wf2 = wf2_pool.tile([128, 2, 640], bf16,
                                                    tag='wf2')
                                nc.sync.dma_start(
                                    out=wf2[:],
                                    in_=wfc2_d[f2 * 256:(f2 + 1) * 256,
                                               clo:chi]
                                    .rearrange('(u p) c -> p u c', p=128))
                                for u in range(2):
                                    fi = 2 * f2 + u
                                    st = (fi == 0)
                                    sp_ = (fi == F // 128 - 1)
                                    for i in range(4):
                                        tt = 4 * ttg + i
                                        lhsT = h2T[:, fi,
                                                   tt * 128:(tt + 1) * 128]
                                        nc.tensor.matmul(
                                            yps[i][:, 0:512], lhsT,
                                            wf2[:, u, 0:512],
                                            start=st, stop=sp_)
                                        nc.tensor.matmul(
                                            yps[i][:, 512:640], lhsT,
                                            wf2[:, u, 512:640],
                                            start=st, stop=sp_)
                            for i in range(4):
                                tt = 4 * ttg + i
                                yo = d_t.tile([128, 640], f32, tag='yo')
                                nc.vector.scalar_tensor_tensor(
                                    out=yo[:], in0=yps[i][:],
                                    scalar=rinv_sb[:, tt:tt + 1],
                                    in1=x2ap(tt)[:, clo:chi],
                                    op0=OP.mult, op1=OP.add)
                                nc.sync.dma_start(
                                    out=y_d[tt * 128:(tt + 1) * 128,
                                            clo:chi],
                                    in_=yo[:])

    nc.compile()
    return nc


_CACHE = {}


def _get_nc(t_len=T):
    if t_len not in _CACHE:
        _CACHE[t_len] = build_nc(t_len)
    return _CACHE[t_len]


def make_in_maps(x, rotary_pos_emb, ln1_w, w_qkv, qn_w, kn_w, w_out, ln2_w,
                 w_fc1, w_fc2, t_len=T):
    """Host-side sharding prep. Returns list of per-core input dicts."""
    x = np.asarray(x, np.float32)
    rot = np.asarray(rotary_pos_emb, np.float32)
    cos = np.cos(rot).astype(np.float32)
    sin = np.sin(rot).astype(np.float32)
    sinneg = np.concatenate([-sin[:, :64], sin[:, :64]], axis=-1)
    qn = np.asarray(qn_w, np.float32)
    kn = np.asarray(kn_w, np.float32)
    cossin = np.stack([cos * qn, sinneg * qn, cos * kn, sinneg * kn],
                      axis=1).reshape(T, 4 * D).astype(ml_dtypes.bfloat16)
    cossin = np.ascontiguousarray(cossin)
    w_qkv_f = (np.asarray(w_qkv, np.float32) * 1024.0
               * np.asarray(ln1_w, np.float32)[:, None]).reshape(C, 3, H, D)
    w_fc1_f = (np.asarray(w_fc1, np.float32)
               * np.asarray(ln2_w, np.float32)[:, None]
               ).astype(ml_dtypes.bfloat16)
    w_fc2_b = np.asarray(w_fc2, np.float32).astype(ml_dtypes.bfloat16)
    # attn carries a *32 scale; w_out carries *1024 fp8 scale (undone by
    # the 2^-15 psum copy scale on-core)
    wo = (np.asarray(w_out, np.float32) * 32768.0 / QSCALE).reshape(H, D, C)

    # tokens owned per rank within a pair: rank0 tiles {0-3, 8-11},
    # rank1 tiles {4-7, 12-15} (tile = 128 tokens)
    halves = [np.r_[0:512, 1024:1536], np.r_[512:1024, 1536:2048]]

    in_maps = []
    for c in range(N_CORES):
        b, hg = c // 2, c % 2
        heads = slice(hg * HPC, (hg + 1) * HPC)
        wq = np.ascontiguousarray(
            w_qkv_f[:, :, heads, :].reshape(C, 3 * CPC)
        ).astype(ml_dtypes.float8_e4m3)
        w_outp = np.ascontiguousarray(
            wo[heads].reshape(CPC, C)).astype(ml_dtypes.float8_e4m3)
        xb = x[b]
        in_maps.append({
            'xsq': np.ascontiguousarray(xb).astype(ml_dtypes.bfloat16),
            'xT': np.ascontiguousarray(
                xb.T * QSCALE).astype(ml_dtypes.float8_e4m3),
            'xh': np.ascontiguousarray(xb[halves[hg]]),
            'w_qkv': wq,
            'cossin': cossin,
            'w_out': w_outp,
            'w_fc1': np.ascontiguousarray(w_fc1_f),
            'w_fc2': np.ascontiguousarray(w_fc2_b),
        })
    return in_maps


def assemble_output(results, t_len=T):
    halves = [np.r_[0:512, 1024:1536], np.r_[512:1024, 1536:2048]]
    out = np.zeros((B, t_len, C), np.float32)
    for c in range(N_CORES):
        b, hg = c // 2, c % 2
        out[b, halves[hg]] = results[c]['y']
    return out


def kernel(**inputs):
    nc = _get_nc(T)
    in_maps = make_in_maps(**inputs)
    res = bass_utils.run_bass_kernel_spmd(nc, in_maps,
                                          core_ids=list(range(N_CORES)))
    return assemble_output(res.results)
